# revision 46
# baseline (speedup 1.0000x reference)
"""AttnBlock for Trainium2, 8 NeuronCores — fp8e4 DoubleRow rewrite (v9).

Sharding: core i = (batch i//2, query-half i%2). Full K/V per core, no
collectives. One program for all cores: odd cores get the image columns
rolled by 2048 (attention is permutation-equivariant over key positions;
GroupNorm stats are order-invariant), so every core computes queries 0..2047
of its (possibly rolled) image.

Math (exact rearrangement of the reference):
  GroupNorm h = scale*x + shift; scale is folded into the fp8 conv WEIGHTS
  on device (w' = fp8(w8 * scale_cin)), so the x->fp8 cast needs no scale
  and runs chunk-by-chunk behind the input DMA, before stats complete.
  shift contributions: k-conv -> constant along m, dropped with bk (softmax
  invariant); v-conv -> v0 = Wv shift + bv contributes Wo v0 to every output
  (softmax rows sum to 1) -> fbias = Wo v0 + bo, pre-added into x in place;
  q-conv -> q0 = Wq shift + bq added in the q epilogue. 1/sqrt(C) is applied
  in the k epilogue. Mg carries 1/GS from the host so group sums emerge as
  [mean, E[x^2]] directly.

Pipeline (cost-model driven; all numbers per core):
  - Phase 1 (~27us, DMA-bound): x streams in 512-col chunks; bn_stats on
    DVE; ONE fp8 copy of x is cast in v-conv stationary layout
    [128, j, mt, ctp, 128] (Act 3/4, Pool 1/4). The k/q convs read their
    MOVING operand from the same tile through a strided view, so the second
    fp8 copy of the old design is gone. A dummy Sqrt at t~0 keeps the
    activation-table load off the stats chain.
  - Convs (~24us): [128,4,256] PSUM ring (4 bufs = all 8 banks), whole-tile
    [128,1024] PSUM->fp8 epilogues alternating Act/DVE (GPSIMD has no PSUM
    port). q convs for the first n-chunk lead; the last three m-chunks'
    k/v epilogues are deferred into attention where the engines have slack.
    The v0/fbias matvecs run mid-conv (off the stats head chain AND off
    the ring tail that the attention pools' bank-reuse waits on).
  - Attention (~73us, jointly exp/PE-bound): per m-chunk, 8 score matmuls
    -> ONE exp(P) [128,4,256] on Act -> PV+den DoubleRow matmuls deferred
    THREE chunks (two for the final chunk, keeping the drain short) so the
    exp engine+sem latency never blocks the PE and the trailing blocks
    spread evenly over the next chunk's first steps. The four
    PV accumulators share 2 PSUM banks as concurrently-open accumulation
    groups (HW start_tensor_calc zeroes only the written region; the sim's
    one-group-per-bank check is skipped). Each chunk's trailing PV pair,
    reciprocal, and softmax divide (one stride-0-broadcast DVE op per half)
    are carried into the next chunk's first steps; proj+residual tails for
    chunk i-1 run at steps 3/5/7 of chunk i. PSUM: s x2 (4 banks) + packed
    PV (2) + den (1) + proj scratch (1).
  - Drain: the freed PV regions double as four independent proj scratches
    (no write-after-read ladder); one wide residual-add and one store.

Known-good invariants learned the hard way:
  - DoubleRow STATIONARY needs its row-pair blocks contiguous in SBUF;
    moving operands tolerate arbitrary strides.
  - Concurrent PSUM accumulation groups in one bank work when they start
    together (PV packing), but a transient group start/stopping while
    another group is mid-accumulation in the same bank corrupts it (the
    den+proj bank-sharing experiment).
  - Pool tiles capture their WAR readers at allocation time: allocate a
    pool generation only after every reader of the previous generation has
    been emitted (the carried-divide race).
"""

import math
import os
import sys

sys.path.insert(0, "/opt/trn_rl_repo")

import numpy as np
import ml_dtypes

import concourse.bacc as bacc
import concourse.bass as bass
import concourse.mybir as mybir
import concourse.tile as tile
from concourse.bass_utils import run_bass_kernel_spmd

F32 = mybir.dt.float32
F32R = mybir.dt.float32r
FP8 = mybir.dt.float8e4
DR = mybir.MatmulPerfMode.DoubleRow
MULT = mybir.AluOpType.mult
ADD = mybir.AluOpType.add
SUB = mybir.AluOpType.subtract
EXP = mybir.ActivationFunctionType.Exp
IDENT = mybir.ActivationFunctionType.Identity
COPY = mybir.ActivationFunctionType.Copy
SQRT = mybir.ActivationFunctionType.Sqrt

B, C, H, W = 4, 512, 64, 64
HW = H * W
G = 32
GS = C // G
NQ = HW // 2
EPS = 1e-5
N_CORES = 8
CT = C // 128
MC = HW // 512          # 8 m-chunks
NC = NQ // 256          # 8 n-chunks
INV_SQRT_C = 1.0 / math.sqrt(C)

LAST_RESULTS = None


def _build():
    nc = bacc.Bacc("TRN2", target_bir_lowering=False, debug=False)

    x_d = nc.dram_tensor("x_img", [C, HW], F32R, kind="ExternalInput").ap()
    w_st = {n: nc.dram_tensor(n, [128, 2, 4, 2, 128], FP8, kind="ExternalInput").ap()
            for n in ("wq8", "wk8", "wo8")}
    wv8m_d = nc.dram_tensor("wv8m", [128, 2, 2, 512], FP8, kind="ExternalInput").ap()
    mg_d = nc.dram_tensor("Mg", [C, G], F32, kind="ExternalInput").ap()
    m2_d = nc.dram_tensor("M2", [G, C], F32, kind="ExternalInput").ap()
    # gamma, beta, bq, bv, bo packed as one [5, C] tensor -> [128, 5, CT] cols
    v5_d = nc.dram_tensor("vec5", [5, C], F32, kind="ExternalInput").ap()
    out_d = nc.dram_tensor("out", [C, NQ], F32, kind="ExternalOutput").ap()

    rx = x_d.rearrange("(t p) m -> p t m", p=128)
    rout = out_d.rearrange("(t p) n -> p t n", p=128)

    with tile.TileContext(nc) as tc:
        with (
            tc.tile_pool(name="singles", bufs=1) as singles,
            tc.tile_pool(name="statp", bufs=3) as statp,
            tc.tile_pool(name="p8p", bufs=6) as p8p,
            tc.tile_pool(name="ao8p", bufs=3) as ao8p,
            tc.tile_pool(name="otp", bufs=3) as otp,
        ):
            # ------ pass 1: x DMA first (the big transfer paces everything);
            # per chunk: bn_stats (DVE only) + x8 cast (Act 3/4, Pool 1/4)
            x_t = singles.tile([128, CT, HW], F32R, tag="x_t")
            # one stats slot per DMA sub-piece: c0 halves(2) + c1-4 full(4)
            # + c5 halves(2) + c6,c7 quarters(8) = 16 per channel-tile
            stats_all = singles.tile([128, CT, 16, 6], F32, tag="stats_all")
            # Single fp8 copy of x in v-conv stationary layout
            # [128, j, mt, ctp, 128] (ctp pair blocks contiguous, as DoubleRow
            # stationary requires). The k/q convs read their MOVING operand
            # from the same tile through a strided [128, ctp, mt, 128] view.
            x8v = singles.tile([128, 2, HW // 128, 2, 128], FP8, tag="x8v")
            # The DMA piece size shrinks toward the end of the stream so the
            # final chunks' bn_stats interleave with their own arrival rather
            # than queueing serially on DVE after the DMA finishes (bn_stats
            # are sufficient statistics, unequal segments aggregate exactly).
            # Chunk 0 is also split so descriptor-generation latency doesn't
            # delay first bytes.
            PIECES = [2, 1, 1, 1, 1, 2, 4, 4]
            slot = 0
            for mc in range(MC):
                npc = PIECES[mc]
                w = 512 // npc
                for pc in range(npc):
                    ps_ = slice(mc * 512 + pc * w, mc * 512 + (pc + 1) * w)
                    nc.sync.dma_start(out=x_t[:, :, ps_], in_=rx[:, :, ps_])
                    for t in range(CT):
                        nc.vector.bn_stats(out=stats_all[:, t, slot, :],
                                           in_=x_t[:, t, ps_])
                    slot += 1
                if mc % 2 == 1:
                    m2s = slice((mc - 1) * 512, (mc + 1) * 512)
                    for t in range(CT):
                        dst = x8v[:, t // 2, 4 * mc - 4:4 * mc + 4, t % 2, :]
                        src = x_t[:, t, m2s].rearrange("p (mt m) -> p mt m",
                                                       m=128)
                        if t < 3:
                            nc.scalar.activation(out=dst, in_=src, func=COPY)
                        else:
                            nc.gpsimd.tensor_copy(out=dst, in_=src)

            # ---------------- constants / small loads ----------------
            v5 = singles.tile([128, 5, CT], F32, tag="v5")
            nc.sync.dma_start(out=v5, in_=v5_d.rearrange("v (t p) -> p v t", p=128))
            cols = {n: v5[:, i, :]
                    for i, n in enumerate(("gamma", "beta", "bq", "bv", "bo"))}
            eps_t = singles.tile([G, 1], F32, tag="eps")
            nc.vector.memset(eps_t, EPS)
            # touch Sqrt once at t~0 so its activation table is resident
            # before the GroupNorm stats chain needs it
            warm = singles.tile([1, 1], F32, tag="warm")
            nc.vector.memset(warm, 1.0)
            nc.scalar.activation(out=warm, in_=warm, func=SQRT, bias=0.0,
                                 scale=1.0)
            ones8 = singles.tile([128, 2, 128], FP8, tag="ones8")
            nc.vector.memset(ones8, 1.0)
            Mg = singles.tile([128, CT, G], F32, tag="Mg")
            nc.sync.dma_start(out=Mg, in_=mg_d.rearrange("(t p) g -> p t g", p=128))
            M2 = singles.tile([G, CT, 128], F32, tag="M2")
            nc.sync.dma_start(out=M2, in_=m2_d.rearrange("g (t p) -> g t p", p=128))

            w8 = {}
            for n in ("wq8", "wk8", "wo8"):
                t = singles.tile([128, 2, 4, 2, 128], FP8, tag=n, name=n)
                nc.scalar.dma_start(out=t, in_=w_st[n])
                w8[n] = t
            wv8 = singles.tile([128, 2, 2, 512], FP8, tag="wv8m")
            nc.scalar.dma_start(out=wv8, in_=wv8m_d)

            def x8m(j, blk0, nblk):
                # moving view for k/q convs: [128, ctp, mt, 128]
                return x8v[:, j, blk0:blk0 + nblk, :, :].rearrange(
                    "p mt ctp m -> p ctp mt m")
            mv = statp.tile([128, CT, 2], F32, tag="mv")
            for t in range(CT):
                nc.vector.bn_aggr(out=mv[:, t, :], in_=stats_all[:, t, :, :])
            s_cat = statp.tile([128, CT, 2], F32, tag="s_cat")
            nc.vector.tensor_copy(out=s_cat[:, :, 0:1], in_=mv[:, :, 0:1])
            nc.vector.tensor_tensor(s_cat[:, :, 1:2], mv[:, :, 0:1], mv[:, :, 0:1], MULT)
            nc.vector.tensor_tensor(s_cat[:, :, 1:2], s_cat[:, :, 1:2], mv[:, :, 1:2], ADD)

            k8 = singles.tile([128, 2, HW // 128, 2, 128], FP8, tag="k8")
            vT8 = singles.tile([128, HW // 256, CT, 2, 128], FP8, tag="vT8")
            q8 = singles.tile([128, 2, NC, 2, 256], FP8, tag="q8")

            # ---------------- conv phase: dedicated 6-bank ring ----------------
            with tc.tile_pool(name="ps_cv", bufs=4, space="PSUM") as ps_cv:
                def cvtile(name):
                    return ps_cv.tile([128, 4, 256], F32, tag="cv", name=name)

                # ---- GroupNorm stats -> per-channel scale/shift ----
                # (Mg carries 1/GS from the host, so gsum = [mean_g, E2_g])
                gsum_t = cvtile("gsum")
                gsum_ps = gsum_t.rearrange("p a b -> p (a b)")
                for ct in range(CT):
                    nc.tensor.matmul(gsum_ps[0:G, 0:2], Mg[:, ct, :], s_cat[:, ct, :],
                                     start=(ct == 0), stop=(ct == CT - 1))
                gme = statp.tile([G, 2], F32, tag="gme")
                nc.vector.tensor_copy(out=gme, in_=gsum_ps[0:G, 0:2])
                gvar = statp.tile([G, 1], F32, tag="gvar")
                nc.vector.tensor_tensor(gvar, gme[:, 0:1], gme[:, 0:1], MULT)
                nc.vector.tensor_tensor(gvar, gme[:, 1:2], gvar, SUB)
                grstd = statp.tile([G, 2], F32, tag="grstd")
                nc.scalar.activation(out=gvar, in_=gvar, func=SQRT, bias=eps_t, scale=1.0)
                nc.vector.reciprocal(grstd[:, 0:1], gvar)
                nc.vector.tensor_copy(out=grstd[:, 1:2], in_=gme[:, 0:1])
                # rm: all four [rstd, mean] channel broadcasts in one tile
                rm_pc = statp.tile([128, CT, 2], F32, tag="rm_pc")
                rm_t = cvtile("rm")
                rm_ps = rm_t.rearrange("p a b -> p (a b)")
                for ct in range(CT):
                    nc.tensor.matmul(rm_ps[:, 2 * ct:2 * ct + 2], M2[:, ct, :],
                                     grstd, start=True, stop=True)
                nc.vector.tensor_copy(
                    out=rm_pc,
                    in_=rm_ps[:, 0:2 * CT].rearrange("p (ct two) -> p ct two",
                                                     two=2))
                scale_pc = singles.tile([128, CT], F32, tag="scale_pc")
                shift_pc = singles.tile([128, CT], F32, tag="shift_pc")
                nc.vector.tensor_tensor(scale_pc, cols["gamma"], rm_pc[:, :, 0], MULT)

                # ---- scaled weight copies on Act+DVE (originals stay for
                #      the matvecs, so no WAR chain). q and k first: their
                #      convs lead the PE stream.
                wk8s = singles.tile([128, 2, 4, 2, 128], FP8, tag="wk8s")
                wv8s = singles.tile([128, 2, 2, 512], FP8, tag="wv8s")
                wq8s = singles.tile([128, 2, 4, 2, 128], FP8, tag="wq8s")

                def wscale(dst, src, ct):
                    sc = scale_pc[:, ct:ct + 1]
                    if ct % 2 == 0:
                        nc.scalar.activation(out=dst, in_=src, func=COPY, scale=sc)
                    else:
                        nc.vector.tensor_scalar(dst, src, sc, None, MULT)

                for ct in range(CT):
                    j, p = ct // 2, ct % 2
                    wscale(wq8s[:, j, :, p, :], w8["wq8"][:, j, :, p, :], ct)
                for ct in range(CT):
                    j, p = ct // 2, ct % 2
                    wscale(wk8s[:, j, :, p, :], w8["wk8"][:, j, :, p, :], ct)
                for ct in range(CT):
                    j, p = ct // 2, ct % 2
                    nc.gpsimd.tensor_scalar(wv8s[:, j, p, :], wv8[:, j, p, :],
                                            scale_pc[:, ct:ct + 1], None, MULT)

                nc.vector.tensor_tensor(shift_pc, scale_pc, rm_pc[:, :, 1], MULT)
                nc.vector.tensor_tensor(shift_pc, cols["beta"], shift_pc, SUB)
                shift8 = singles.tile([128, CT, 1], FP8, tag="shift8")
                nc.vector.tensor_scalar_mul(shift8[:, :, 0], shift_pc, 1.0)

                # ---- matvecs on UNscaled weights: q0, v0, fbias ----
                q0col = singles.tile([128, CT], F32, tag="q0col")
                v0col = singles.tile([128, CT], F32, tag="v0col")
                v08 = singles.tile([128, CT, 1], FP8, tag="v08")
                fbias = singles.tile([128, CT], F32, tag="fbias")
                mv_t = cvtile("mv_ps")
                mv_ps = mv_t.rearrange("p a b -> p (a b)")
                for dt in range(CT):
                    for ct in range(CT):
                        nc.tensor.matmul(mv_ps[:, dt:dt + 1],
                                         w8["wq8"][:, ct // 2, dt, ct % 2, :],
                                         shift8[:, ct, :],
                                         start=(ct == 0), stop=(ct == CT - 1))
                nc.vector.tensor_tensor(q0col, mv_ps[:, 0:CT], cols["bq"], ADD)
                # ---- convs; whole-tile [128,1024] epilogues alternate
                #      Act/DVE. q for t=0 first (attention needs q8[0]);
                #      the rest of q after k/v (consumed much later).
                def q_conv(t):
                    for i in range(2):
                        qt = cvtile("qc").rearrange("p a b -> p (a b)") \
                            .rearrange("p (d two n) -> p d two n", d=2, n=256)
                        for d in range(2):
                            dt = 2 * i + d
                            for j in range(2):
                                nc.tensor.matmul(qt[:, d], wq8s[:, j, dt],
                                                 x8m(j, 4 * t, 4),
                                                 start=(j == 0), stop=(j == 1),
                                                 perf_mode=DR)
                        for d in range(2):
                            dt = 2 * i + d
                            dst = q8[:, dt // 2, 2 * t:2 * t + 2, dt % 2, :]
                            if d == 0:
                                nc.scalar.activation(
                                    out=dst, in_=qt[:, d], func=IDENT,
                                    bias=q0col[:, dt:dt + 1], scale=1.0)
                            else:
                                nc.vector.tensor_scalar(
                                    dst, qt[:, d], q0col[:, dt:dt + 1], None, ADD)

                q_conv(0)
                deferred_epis = []

                def k_epi(kt, mc, i):
                    def run():
                        dst = k8[:, i, 4 * mc:4 * mc + 4, :, :]
                        src = kt.rearrange("p d mt m -> p mt d m")
                        if i == 0:
                            nc.scalar.activation(out=dst, in_=src,
                                                 func=COPY, scale=INV_SQRT_C)
                        else:
                            nc.vector.tensor_scalar(dst, src,
                                                    INV_SQRT_C, None, MULT)
                    return run

                def v_epi(vt, h):
                    def run():
                        dst = vT8[:, h, :, :, :]
                        src = vt.rearrange("p g ct m -> p ct g m")
                        if h % 2 == 1 or h == 6:
                            nc.scalar.activation(out=dst, in_=src, func=COPY)
                        else:
                            nc.vector.tensor_copy(out=dst, in_=src)
                    return run

                def fbias_matvecs():
                    # v0/fbias matvecs mid-conv: late enough to stay off the
                    # stats->wk8s head chain, early enough that the attention
                    # pools' bank-reuse WAR does not wait on them
                    mv_t2 = cvtile("mv_ps2")
                    mv_ps2 = mv_t2.rearrange("p a b -> p (a b)")
                    for dt in range(CT):
                        for ct in range(CT):
                            nc.tensor.matmul(mv_ps2[:, dt:dt + 1],
                                             wv8[:, ct // 2, ct % 2,
                                                 dt * 128:(dt + 1) * 128],
                                             shift8[:, ct, :],
                                             start=(ct == 0),
                                             stop=(ct == CT - 1))
                    nc.vector.tensor_tensor(v0col, mv_ps2[:, 0:CT],
                                            cols["bv"], ADD)
                    nc.vector.tensor_scalar_mul(v08[:, :, 0], v0col, 1.0)
                    mv_t3 = cvtile("mv_ps3")
                    mv_ps3 = mv_t3.rearrange("p a b -> p (a b)")
                    for dt in range(CT):
                        for ct in range(CT):
                            nc.tensor.matmul(mv_ps3[:, dt:dt + 1],
                                             w8["wo8"][:, ct // 2, dt,
                                                       ct % 2, :],
                                             v08[:, ct, :],
                                             start=(ct == 0),
                                             stop=(ct == CT - 1))
                    nc.vector.tensor_tensor(fbias, mv_ps3[:, 0:CT],
                                            cols["bo"], ADD)
                    for ct in range(CT):
                        nc.gpsimd.tensor_scalar(
                            x_t[:, ct, 0:NQ], x_t[:, ct, 0:NQ],
                            fbias[:, ct:ct + 1], None, ADD)

                for mc in range(MC):
                    if mc == 4:
                        fbias_matvecs()
                    for i in range(2):          # dt pair (2i, 2i+1)
                        kt = cvtile("kc").rearrange("p a b -> p (a b)") \
                            .rearrange("p (d mt m) -> p d mt m", d=2, m=128)
                        for d in range(2):
                            dt = 2 * i + d
                            for j in range(2):
                                nc.tensor.matmul(kt[:, d], wk8s[:, j, dt],
                                                 x8m(j, 4 * mc, 4),
                                                 start=(j == 0), stop=(j == 1),
                                                 perf_mode=DR)
                        if mc < MC - 3:
                            k_epi(kt, mc, i)()
                        else:
                            deferred_epis.append(k_epi(kt, mc, i))
                    for h in (2 * mc, 2 * mc + 1):  # msub pair (2h, 2h+1)
                        vt = cvtile("vc").rearrange("p a b -> p (a b)") \
                            .rearrange("p (g ct m) -> p g ct m", g=2, m=128)
                        for gi in range(2):
                            g = 2 * h + gi
                            for j in range(2):
                                nc.tensor.matmul(vt[:, gi], x8v[:, j, g],
                                                 wv8s[:, j],
                                                 start=(j == 0), stop=(j == 1),
                                                 perf_mode=DR)
                        if mc < MC - 3:
                            v_epi(vt, h)()
                        else:
                            deferred_epis.append(v_epi(vt, h))
                for t in range(1, CT):
                    q_conv(t)
                # preload the Exp activation table: this dummy exp DEPENDS on
                # the last q8 epilogue, pinning the (implicit) table load to
                # the conv tail where Act idles — an undepended dummy gets
                # scheduled early and steals Act time from conv epilogues
                nc.scalar.activation(out=warm, in_=q8[0:1, 1, NC - 1, 1, 0:1],
                                     func=EXP)


            # ---------------- attention (s x2 + pv + den/proj = 8 banks) ----
            with (
                tc.tile_pool(name="ps_s", bufs=2, space="PSUM") as ps_s,
                tc.tile_pool(name="ps_pv", bufs=1, space="PSUM") as ps_pv,
                tc.tile_pool(name="ps_o", bufs=1, space="PSUM") as ps_o,
            ):
                pending = None  # (nci, ao8)

                def tail_step(dts, ot, pr):
                    pnci, pao8 = pending
                    for dt in dts:
                        pr_ps = pr[:, dt % 2, :]
                        for j in range(2):
                            nc.tensor.matmul(pr_ps, w8["wo8"][:, j, dt],
                                             pao8[:, 2 * j:2 * j + 2, :],
                                             start=(j == 0), stop=(j == 1),
                                             perf_mode=DR,
                                             skip_group_check=True)
                        nc.vector.tensor_tensor(
                            ot[:, dt, :], pr_ps,
                            x_t[:, dt, pnci * 256:(pnci + 1) * 256], ADD)

                def tail_flush(ot):
                    pnci = pending[0]
                    nc.sync.dma_start(
                        out=rout[:, :, pnci * 256:(pnci + 1) * 256], in_=ot)

                def emit_pv(pm, pp, pv, den_ps):
                    # den first: its stop gates the reciprocal, so retiring
                    # it at block start shortens the divide chain
                    for u in range(2):
                        nc.tensor.matmul(
                            den_ps, ones8, pp[:, 2 * u:2 * u + 2, :],
                            start=(pm == 0 and u == 0),
                            stop=(pm == MC - 1 and u == 1), perf_mode=DR,
                            skip_group_check=True)
                        for ct in range(CT):
                            nc.tensor.matmul(
                                pv[ct], vT8[:, 2 * pm + u, ct],
                                pp[:, 2 * u:2 * u + 2, :],
                                start=(pm == 0 and u == 0),
                                stop=(pm == MC - 1 and u == 1),
                                perf_mode=DR, skip_group_check=True)

                def divide(pnci, prev_pv_all, den_ps):
                    # reciprocal + two half-divides (each half releases its
                    # pv regions and unblocks the matching proj j-step)
                    rec = statp.tile([128, 256], F32, tag="rec", name="rec")
                    nc.vector.reciprocal(rec, den_ps)
                    ao8 = ao8p.tile([128, CT, 256], FP8, tag="ao8", name="ao8")
                    rec_h = bass.AP(rec.tensor, rec.offset,
                                    [rec.ap[0], [0, 2], rec.ap[1]])
                    for hh in range(2):
                        nc.vector.tensor_tensor(
                            ao8[:, 2 * hh:2 * hh + 2, :],
                            prev_pv_all[:, 2 * hh:2 * hh + 2, :], rec_h, MULT)
                    return (pnci, ao8)

                prev = None  # (p_tiles, pv_all, den_ps) of nci-1
                prev_defer = None
                for nci in range(NC):
                    # defer-3 spreads each chunk's trailing PV over three
                    # steps of the next chunk; the last chunk stays defer-2
                    # so the drain keeps only two trailing blocks
                    defer = 2 if nci == NC - 1 else 3
                    p_tiles = []
                    pv_all = den_ps = pv = None
                    ot = otp.tile([128, CT, 256], F32, tag="ot", name="ot") \
                        if prev is not None else None
                    pr = ps_o.tile([128, 2, 256], F32, tag="o", name="pr") \
                        if prev is not None else None
                    for mc in range(MC):
                        s_ps = ps_s.tile([128, 4, 256], F32, tag="s", name="s_ps")
                        for msub in range(4):
                            for j in range(2):
                                nc.tensor.matmul(
                                    s_ps[:, msub, :], k8[:, j, 4 * mc + msub],
                                    q8[:, j, nci],
                                    start=(j == 0), stop=(j == 1), perf_mode=DR)
                        p8 = p8p.tile([128, 4, 256], FP8, tag="p8", name="p8")
                        p_tiles.append(p8)
                        nc.scalar.activation(out=p8, in_=s_ps, func=EXP)
                        if nci == 0 and mc >= 2 and deferred_epis:
                            deferred_epis.pop(0)()
                            if deferred_epis:
                                deferred_epis.pop(0)()
                        if prev is not None and mc < prev_defer:
                            # previous chunk's trailing PV + softmax divide
                            pp_, pva_, den_ = prev
                            emit_pv(MC - prev_defer + mc,
                                    pp_[MC - prev_defer + mc],
                                    [pva_[:, ct, :] for ct in range(CT)], den_)
                            if mc == prev_defer - 1:
                                pending = divide(nci - 1, pva_, den_)
                        if mc == defer:
                            # allocate AFTER the previous generation's readers
                            # (trailing PV + divide) are emitted, so the pool
                            # WAR edges cover them
                            pv_all = ps_pv.tile([128, 4, 256], F32, tag="pva",
                                                name="pva")
                            pv = [pv_all[:, ct, :] for ct in range(CT)]
                            den_ps = ps_pv.tile([128, 256], F32, tag="den",
                                                name="den")
                        if mc >= defer:
                            emit_pv(mc - defer, p_tiles[mc - defer], pv, den_ps)
                        if pending is not None and mc >= 4:
                            tail_step([mc - 4], ot, pr)
                            if mc == MC - 1:
                                tail_flush(ot)
                                pending = None
                    prev = (p_tiles, pv_all, den_ps)
                    prev_defer = defer
                # drain: last chunk's trailing PV, divide, proj, store
                pp_, pva_, den_ = prev
                pvl = [pva_[:, ct, :] for ct in range(CT)]
                emit_pv(MC - 2, pp_[MC - 2], pvl, den_)
                emit_pv(MC - 1, pp_[MC - 1], pvl, den_)
                # final drain pipelined by query-halves: divide, proj,
                # residual and store for queries 0:128 flow while 128:256 is
                # still dividing. Per-dt j-pair order and per-region
                # accumulation order are unchanged.
                pnci = NC - 1
                rec = statp.tile([128, 256], F32, tag="rec", name="rec_f")
                nc.vector.reciprocal(rec, den_)
                ao8 = ao8p.tile([128, CT, 256], FP8, tag="ao8", name="ao8_f")
                ot = otp.tile([128, CT, 256], F32, tag="ot", name="ot_f")
                for qh in range(2):
                    qs = slice(128 * qh, 128 * qh + 128)
                    rec_q = bass.AP(rec.tensor, rec.offset + 128 * qh,
                                    [rec.ap[0], [0, CT], [1, 128]])
                    nc.vector.tensor_tensor(ao8[:, :, qs], pva_[:, :, qs],
                                            rec_q, MULT)
                for qh in range(2):
                    qs = slice(128 * qh, 128 * qh + 128)
                    for dt in range(CT):
                        pr_ps = pva_[:, dt, qs]
                        for j in range(2):
                            nc.tensor.matmul(pr_ps, w8["wo8"][:, j, dt],
                                             ao8[:, 2 * j:2 * j + 2, qs],
                                             start=(j == 0), stop=(j == 1),
                                             perf_mode=DR,
                                             skip_group_check=True)
                    nc.vector.tensor_tensor(
                        ot[:, :, qs], pva_[:, :, qs],
                        x_t[:, :, pnci * 256 + 128 * qh:
                            pnci * 256 + 128 * qh + 128], ADD)
                    nc.sync.dma_start(
                        out=rout[:, :, pnci * 256 + 128 * qh:
                                 pnci * 256 + 128 * qh + 128],
                        in_=ot[:, :, qs])
    nc.finalize()
    return nc


_NC_CACHE = {}


def _get_nc():
    if "nc" not in _NC_CACHE:
        _NC_CACHE["nc"] = _build()
    return _NC_CACHE["nc"]


def _prep_stationary(w):
    # w: [cout, cin] conv weight -> stationary DR layout [p, j, dt, ctp, m]
    wT = np.ascontiguousarray(w.T)                      # [cin, cout]
    arr = wT.reshape(2, 2, 128, 4, 128)                  # [j, ctp, p, dt, m]
    arr = np.transpose(arr, (2, 0, 3, 1, 4))             # [p, j, dt, ctp, m]
    return np.ascontiguousarray(arr).astype(ml_dtypes.float8_e4m3)


def _prep_moving(w):
    # w: [cout, cin] -> moving DR layout [p, j, ctp, cout]
    wT = np.ascontiguousarray(w.T)                      # [cin, cout]
    arr = wT.reshape(2, 2, 128, 512)                     # [j, ctp, p, cout]
    arr = np.transpose(arr, (2, 0, 1, 3))                # [p, j, ctp, cout]
    return np.ascontiguousarray(arr).astype(ml_dtypes.float8_e4m3)


def kernel(**inputs):
    x = np.ascontiguousarray(np.asarray(inputs["x"], dtype=np.float32))
    gamma = np.asarray(inputs["gamma"], np.float32)
    beta = np.asarray(inputs["beta"], np.float32)
    w = {n: np.asarray(inputs[n], np.float32) for n in ("wq", "wk", "wv", "wo")}
    b = {n: np.asarray(inputs[n], np.float32) for n in ("bq", "bk", "bv", "bo")}

    mg_np = np.zeros((C, G), np.float32)
    mg_np[np.arange(C), np.arange(C) // GS] = 1.0 / GS
    common = {
        "Mg": mg_np,
        "M2": np.ascontiguousarray((mg_np != 0).astype(np.float32).T),
        "wq8": _prep_stationary(w["wq"]),
        "wk8": _prep_stationary(w["wk"]),
        "wo8": _prep_stationary(w["wo"]),
        "wv8m": _prep_moving(w["wv"]),
        "vec5": np.ascontiguousarray(
            np.stack([gamma, beta, b["bq"], b["bv"], b["bo"]])),
    }
    in_maps = []
    for core in range(N_CORES):
        bi, ch = divmod(core, 2)
        xi = x[bi].reshape(C, HW)
        if ch:
            xi = np.roll(xi, -NQ, axis=1)
        m = dict(common)
        m["x_img"] = np.ascontiguousarray(xi)
        in_maps.append(m)

    want_trace = bool(int(os.environ.get("KTRACE", "0")))
    if not want_trace:
        os.environ["BASS_NEVER_TRACE"] = "1"
    global LAST_RESULTS
    LAST_RESULTS = run_bass_kernel_spmd(
        _get_nc(), in_maps, core_ids=list(range(N_CORES)), trace=want_trace)
    full = np.empty((B, C, HW), np.float32)
    for core in range(N_CORES):
        bi, ch = divmod(core, 2)
        full[bi][:, ch * NQ:(ch + 1) * NQ] = LAST_RESULTS.results[core]["out"]
    return full.reshape(B, C, H, W)


# revision 47
# speedup vs baseline: 1.0024x; 1.0024x over previous
"""AttnBlock for Trainium2, 8 NeuronCores — fp8e4 DoubleRow rewrite (v9).

Sharding: core i = (batch i//2, query-half i%2). Full K/V per core, no
collectives. One program for all cores: odd cores get the image columns
rolled by 2048 (attention is permutation-equivariant over key positions;
GroupNorm stats are order-invariant), so every core computes queries 0..2047
of its (possibly rolled) image.

Math (exact rearrangement of the reference):
  GroupNorm h = scale*x + shift; scale is folded into the fp8 conv WEIGHTS
  on device (w' = fp8(w8 * scale_cin)), so the x->fp8 cast needs no scale
  and runs chunk-by-chunk behind the input DMA, before stats complete.
  shift contributions: k-conv -> constant along m, dropped with bk (softmax
  invariant); v-conv -> v0 = Wv shift + bv contributes Wo v0 to every output
  (softmax rows sum to 1) -> fbias = Wo v0 + bo, pre-added into x in place;
  q-conv -> q0 = Wq shift + bq added in the q epilogue. 1/sqrt(C) is applied
  in the k epilogue. Mg carries 1/GS from the host so group sums emerge as
  [mean, E[x^2]] directly.

Pipeline (cost-model driven; all numbers per core):
  - Phase 1 (~27us, DMA-bound): x streams in 512-col chunks; bn_stats on
    DVE; ONE fp8 copy of x is cast in v-conv stationary layout
    [128, j, mt, ctp, 128] (Act 3/4, Pool 1/4). The k/q convs read their
    MOVING operand from the same tile through a strided view, so the second
    fp8 copy of the old design is gone. A dummy Sqrt at t~0 keeps the
    activation-table load off the stats chain.
  - Convs (~24us): [128,4,256] PSUM ring (4 bufs = all 8 banks), whole-tile
    [128,1024] PSUM->fp8 epilogues alternating Act/DVE (GPSIMD has no PSUM
    port). q convs for the first n-chunk lead; the last three m-chunks'
    k/v epilogues are deferred into attention where the engines have slack.
    The v0/fbias matvecs run mid-conv (off the stats head chain AND off
    the ring tail that the attention pools' bank-reuse waits on).
  - Attention (~73us, jointly exp/PE-bound): per m-chunk, 8 score matmuls
    -> ONE exp(P) [128,4,256] on Act -> PV+den DoubleRow matmuls deferred
    THREE chunks (two for the final chunk, keeping the drain short) so the
    exp engine+sem latency never blocks the PE and the trailing blocks
    spread evenly over the next chunk's first steps. The four
    PV accumulators share 2 PSUM banks as concurrently-open accumulation
    groups (HW start_tensor_calc zeroes only the written region; the sim's
    one-group-per-bank check is skipped). Each chunk's trailing PV pair,
    reciprocal, and softmax divide (one stride-0-broadcast DVE op per half)
    are carried into the next chunk's first steps; proj+residual tails for
    chunk i-1 run at steps 3/5/7 of chunk i. PSUM: s x2 (4 banks) + packed
    PV (2) + den (1) + proj scratch (1).
  - Drain: the freed PV regions double as four independent proj scratches
    (no write-after-read ladder); one wide residual-add and one store.

Known-good invariants learned the hard way:
  - DoubleRow STATIONARY needs its row-pair blocks contiguous in SBUF;
    moving operands tolerate arbitrary strides.
  - Concurrent PSUM accumulation groups in one bank work when they start
    together (PV packing), but a transient group start/stopping while
    another group is mid-accumulation in the same bank corrupts it (the
    den+proj bank-sharing experiment).
  - Pool tiles capture their WAR readers at allocation time: allocate a
    pool generation only after every reader of the previous generation has
    been emitted (the carried-divide race).
"""

import math
import os
import sys

sys.path.insert(0, "/opt/trn_rl_repo")

import numpy as np
import ml_dtypes

import concourse.bacc as bacc
import concourse.bass as bass
import concourse.mybir as mybir
import concourse.tile as tile
from concourse.bass_utils import run_bass_kernel_spmd

F32 = mybir.dt.float32
F32R = mybir.dt.float32r
FP8 = mybir.dt.float8e4
DR = mybir.MatmulPerfMode.DoubleRow
MULT = mybir.AluOpType.mult
ADD = mybir.AluOpType.add
SUB = mybir.AluOpType.subtract
EXP = mybir.ActivationFunctionType.Exp
IDENT = mybir.ActivationFunctionType.Identity
COPY = mybir.ActivationFunctionType.Copy
SQRT = mybir.ActivationFunctionType.Sqrt

B, C, H, W = 4, 512, 64, 64
HW = H * W
G = 32
GS = C // G
NQ = HW // 2
EPS = 1e-5
N_CORES = 8
CT = C // 128
MC = HW // 512          # 8 m-chunks
NC = NQ // 256          # 8 n-chunks
INV_SQRT_C = 1.0 / math.sqrt(C)

LAST_RESULTS = None


def _build():
    nc = bacc.Bacc("TRN2", target_bir_lowering=False, debug=False)

    x_d = nc.dram_tensor("x_img", [C, HW], F32R, kind="ExternalInput").ap()
    w_st = {n: nc.dram_tensor(n, [128, 2, 4, 2, 128], FP8, kind="ExternalInput").ap()
            for n in ("wq8", "wk8", "wo8")}
    wv8m_d = nc.dram_tensor("wv8m", [128, 2, 2, 512], FP8, kind="ExternalInput").ap()
    mg_d = nc.dram_tensor("Mg", [C, G], F32, kind="ExternalInput").ap()
    m2_d = nc.dram_tensor("M2", [G, C], F32, kind="ExternalInput").ap()
    # gamma, beta, bq, bv, bo packed as one [5, C] tensor -> [128, 5, CT] cols
    v5_d = nc.dram_tensor("vec5", [5, C], F32, kind="ExternalInput").ap()
    out_d = nc.dram_tensor("out", [C, NQ], F32, kind="ExternalOutput").ap()

    rx = x_d.rearrange("(t p) m -> p t m", p=128)
    rout = out_d.rearrange("(t p) n -> p t n", p=128)

    with tile.TileContext(nc) as tc:
        with (
            tc.tile_pool(name="singles", bufs=1) as singles,
            tc.tile_pool(name="statp", bufs=3) as statp,
            tc.tile_pool(name="p8p", bufs=6) as p8p,
            tc.tile_pool(name="ao8p", bufs=3) as ao8p,
            tc.tile_pool(name="otp", bufs=3) as otp,
        ):
            # ------ pass 1: x DMA first (the big transfer paces everything);
            # per chunk: bn_stats (DVE only) + x8 cast (Act 3/4, Pool 1/4)
            x_t = singles.tile([128, CT, HW], F32R, tag="x_t")
            stats_all = singles.tile([128, CT, MC + 1, 6], F32, tag="stats_all")
            # Single fp8 copy of x in v-conv stationary layout
            # [128, j, mt, ctp, 128] (ctp pair blocks contiguous, as DoubleRow
            # stationary requires). The k/q convs read their MOVING operand
            # from the same tile through a strided [128, ctp, mt, 128] view.
            x8v = singles.tile([128, 2, HW // 128, 2, 128], FP8, tag="x8v")
            for mc in range(MC):
                ms = slice(mc * 512, (mc + 1) * 512)
                if mc in (0, MC - 1):
                    # split the first chunk (descriptor-generation latency
                    # shouldn't delay first bytes) and the last chunk (its
                    # first-half bn_stats overlap the second half's DMA,
                    # shortening the stats tail; bn_stats are sufficient
                    # statistics so unequal segments aggregate exactly)
                    h0 = slice(mc * 512, mc * 512 + 256)
                    h1 = slice(mc * 512 + 256, (mc + 1) * 512)
                    nc.sync.dma_start(out=x_t[:, :, h0], in_=rx[:, :, h0])
                    nc.sync.dma_start(out=x_t[:, :, h1], in_=rx[:, :, h1])
                else:
                    nc.sync.dma_start(out=x_t[:, :, ms], in_=rx[:, :, ms])
                if mc == MC - 1:
                    h0 = slice(mc * 512, mc * 512 + 256)
                    h1 = slice(mc * 512 + 256, (mc + 1) * 512)
                    for t in range(CT):
                        nc.vector.bn_stats(out=stats_all[:, t, mc, :],
                                           in_=x_t[:, t, h0])
                        nc.vector.bn_stats(out=stats_all[:, t, MC, :],
                                           in_=x_t[:, t, h1])
                else:
                    for t in range(CT):
                        nc.vector.bn_stats(out=stats_all[:, t, mc, :],
                                           in_=x_t[:, t, ms])
                if mc % 2 == 1:
                    m2s = slice((mc - 1) * 512, (mc + 1) * 512)
                    for t in range(CT):
                        dst = x8v[:, t // 2, 4 * mc - 4:4 * mc + 4, t % 2, :]
                        src = x_t[:, t, m2s].rearrange("p (mt m) -> p mt m",
                                                       m=128)
                        if t < 3:
                            nc.scalar.activation(out=dst, in_=src, func=COPY)
                        else:
                            nc.gpsimd.tensor_copy(out=dst, in_=src)

            # ---------------- constants / small loads ----------------
            v5 = singles.tile([128, 5, CT], F32, tag="v5")
            nc.sync.dma_start(out=v5, in_=v5_d.rearrange("v (t p) -> p v t", p=128))
            cols = {n: v5[:, i, :]
                    for i, n in enumerate(("gamma", "beta", "bq", "bv", "bo"))}
            eps_t = singles.tile([G, 1], F32, tag="eps")
            nc.vector.memset(eps_t, EPS)
            # touch Sqrt once at t~0 so its activation table is resident
            # before the GroupNorm stats chain needs it
            warm = singles.tile([1, 1], F32, tag="warm")
            nc.vector.memset(warm, 1.0)
            nc.scalar.activation(out=warm, in_=warm, func=SQRT, bias=0.0,
                                 scale=1.0)
            ones8 = singles.tile([128, 2, 128], FP8, tag="ones8")
            nc.vector.memset(ones8, 1.0)
            Mg = singles.tile([128, CT, G], F32, tag="Mg")
            nc.sync.dma_start(out=Mg, in_=mg_d.rearrange("(t p) g -> p t g", p=128))
            M2 = singles.tile([G, CT, 128], F32, tag="M2")
            nc.sync.dma_start(out=M2, in_=m2_d.rearrange("g (t p) -> g t p", p=128))

            w8 = {}
            for n in ("wq8", "wk8", "wo8"):
                t = singles.tile([128, 2, 4, 2, 128], FP8, tag=n, name=n)
                nc.scalar.dma_start(out=t, in_=w_st[n])
                w8[n] = t
            wv8 = singles.tile([128, 2, 2, 512], FP8, tag="wv8m")
            nc.scalar.dma_start(out=wv8, in_=wv8m_d)

            def x8m(j, blk0, nblk):
                # moving view for k/q convs: [128, ctp, mt, 128]
                return x8v[:, j, blk0:blk0 + nblk, :, :].rearrange(
                    "p mt ctp m -> p ctp mt m")
            mv = statp.tile([128, CT, 2], F32, tag="mv")
            for t in range(CT):
                nc.vector.bn_aggr(out=mv[:, t, :], in_=stats_all[:, t, :, :])
            s_cat = statp.tile([128, CT, 2], F32, tag="s_cat")
            nc.vector.tensor_copy(out=s_cat[:, :, 0:1], in_=mv[:, :, 0:1])
            nc.vector.tensor_tensor(s_cat[:, :, 1:2], mv[:, :, 0:1], mv[:, :, 0:1], MULT)
            nc.vector.tensor_tensor(s_cat[:, :, 1:2], s_cat[:, :, 1:2], mv[:, :, 1:2], ADD)

            k8 = singles.tile([128, 2, HW // 128, 2, 128], FP8, tag="k8")
            vT8 = singles.tile([128, HW // 256, CT, 2, 128], FP8, tag="vT8")
            q8 = singles.tile([128, 2, NC, 2, 256], FP8, tag="q8")

            # ---------------- conv phase: dedicated 6-bank ring ----------------
            with tc.tile_pool(name="ps_cv", bufs=4, space="PSUM") as ps_cv:
                def cvtile(name):
                    return ps_cv.tile([128, 4, 256], F32, tag="cv", name=name)

                # ---- GroupNorm stats -> per-channel scale/shift ----
                # (Mg carries 1/GS from the host, so gsum = [mean_g, E2_g])
                gsum_t = cvtile("gsum")
                gsum_ps = gsum_t.rearrange("p a b -> p (a b)")
                for ct in range(CT):
                    nc.tensor.matmul(gsum_ps[0:G, 0:2], Mg[:, ct, :], s_cat[:, ct, :],
                                     start=(ct == 0), stop=(ct == CT - 1))
                gme = statp.tile([G, 2], F32, tag="gme")
                nc.vector.tensor_copy(out=gme, in_=gsum_ps[0:G, 0:2])
                gvar = statp.tile([G, 1], F32, tag="gvar")
                nc.vector.tensor_tensor(gvar, gme[:, 0:1], gme[:, 0:1], MULT)
                nc.vector.tensor_tensor(gvar, gme[:, 1:2], gvar, SUB)
                grstd = statp.tile([G, 2], F32, tag="grstd")
                nc.scalar.activation(out=gvar, in_=gvar, func=SQRT, bias=eps_t, scale=1.0)
                nc.vector.reciprocal(grstd[:, 0:1], gvar)
                nc.vector.tensor_copy(out=grstd[:, 1:2], in_=gme[:, 0:1])
                # rm: all four [rstd, mean] channel broadcasts in one tile
                rm_pc = statp.tile([128, CT, 2], F32, tag="rm_pc")
                rm_t = cvtile("rm")
                rm_ps = rm_t.rearrange("p a b -> p (a b)")
                for ct in range(CT):
                    nc.tensor.matmul(rm_ps[:, 2 * ct:2 * ct + 2], M2[:, ct, :],
                                     grstd, start=True, stop=True)
                nc.vector.tensor_copy(
                    out=rm_pc,
                    in_=rm_ps[:, 0:2 * CT].rearrange("p (ct two) -> p ct two",
                                                     two=2))
                scale_pc = singles.tile([128, CT], F32, tag="scale_pc")
                shift_pc = singles.tile([128, CT], F32, tag="shift_pc")
                nc.vector.tensor_tensor(scale_pc, cols["gamma"], rm_pc[:, :, 0], MULT)

                # ---- scaled weight copies on Act+DVE (originals stay for
                #      the matvecs, so no WAR chain). q and k first: their
                #      convs lead the PE stream.
                wk8s = singles.tile([128, 2, 4, 2, 128], FP8, tag="wk8s")
                wv8s = singles.tile([128, 2, 2, 512], FP8, tag="wv8s")
                wq8s = singles.tile([128, 2, 4, 2, 128], FP8, tag="wq8s")

                def wscale(dst, src, ct):
                    sc = scale_pc[:, ct:ct + 1]
                    if ct % 2 == 0:
                        nc.scalar.activation(out=dst, in_=src, func=COPY, scale=sc)
                    else:
                        nc.vector.tensor_scalar(dst, src, sc, None, MULT)

                for ct in range(CT):
                    j, p = ct // 2, ct % 2
                    wscale(wq8s[:, j, :, p, :], w8["wq8"][:, j, :, p, :], ct)
                for ct in range(CT):
                    j, p = ct // 2, ct % 2
                    wscale(wk8s[:, j, :, p, :], w8["wk8"][:, j, :, p, :], ct)
                for ct in range(CT):
                    j, p = ct // 2, ct % 2
                    nc.gpsimd.tensor_scalar(wv8s[:, j, p, :], wv8[:, j, p, :],
                                            scale_pc[:, ct:ct + 1], None, MULT)

                nc.vector.tensor_tensor(shift_pc, scale_pc, rm_pc[:, :, 1], MULT)
                nc.vector.tensor_tensor(shift_pc, cols["beta"], shift_pc, SUB)
                shift8 = singles.tile([128, CT, 1], FP8, tag="shift8")
                nc.vector.tensor_scalar_mul(shift8[:, :, 0], shift_pc, 1.0)

                # ---- matvecs on UNscaled weights: q0, v0, fbias ----
                q0col = singles.tile([128, CT], F32, tag="q0col")
                v0col = singles.tile([128, CT], F32, tag="v0col")
                v08 = singles.tile([128, CT, 1], FP8, tag="v08")
                fbias = singles.tile([128, CT], F32, tag="fbias")
                mv_t = cvtile("mv_ps")
                mv_ps = mv_t.rearrange("p a b -> p (a b)")
                for dt in range(CT):
                    for ct in range(CT):
                        nc.tensor.matmul(mv_ps[:, dt:dt + 1],
                                         w8["wq8"][:, ct // 2, dt, ct % 2, :],
                                         shift8[:, ct, :],
                                         start=(ct == 0), stop=(ct == CT - 1))
                nc.vector.tensor_tensor(q0col, mv_ps[:, 0:CT], cols["bq"], ADD)
                # ---- convs; whole-tile [128,1024] epilogues alternate
                #      Act/DVE. q for t=0 first (attention needs q8[0]);
                #      the rest of q after k/v (consumed much later).
                def q_conv(t):
                    for i in range(2):
                        qt = cvtile("qc").rearrange("p a b -> p (a b)") \
                            .rearrange("p (d two n) -> p d two n", d=2, n=256)
                        for d in range(2):
                            dt = 2 * i + d
                            for j in range(2):
                                nc.tensor.matmul(qt[:, d], wq8s[:, j, dt],
                                                 x8m(j, 4 * t, 4),
                                                 start=(j == 0), stop=(j == 1),
                                                 perf_mode=DR)
                        for d in range(2):
                            dt = 2 * i + d
                            dst = q8[:, dt // 2, 2 * t:2 * t + 2, dt % 2, :]
                            if d == 0:
                                nc.scalar.activation(
                                    out=dst, in_=qt[:, d], func=IDENT,
                                    bias=q0col[:, dt:dt + 1], scale=1.0)
                            else:
                                nc.vector.tensor_scalar(
                                    dst, qt[:, d], q0col[:, dt:dt + 1], None, ADD)

                q_conv(0)
                deferred_epis = []

                def k_epi(kt, mc, i):
                    def run():
                        dst = k8[:, i, 4 * mc:4 * mc + 4, :, :]
                        src = kt.rearrange("p d mt m -> p mt d m")
                        if i == 0:
                            nc.scalar.activation(out=dst, in_=src,
                                                 func=COPY, scale=INV_SQRT_C)
                        else:
                            nc.vector.tensor_scalar(dst, src,
                                                    INV_SQRT_C, None, MULT)
                    return run

                def v_epi(vt, h):
                    def run():
                        dst = vT8[:, h, :, :, :]
                        src = vt.rearrange("p g ct m -> p ct g m")
                        if h % 2 == 1 or h == 6:
                            nc.scalar.activation(out=dst, in_=src, func=COPY)
                        else:
                            nc.vector.tensor_copy(out=dst, in_=src)
                    return run

                def fbias_matvecs():
                    # v0/fbias matvecs mid-conv: late enough to stay off the
                    # stats->wk8s head chain, early enough that the attention
                    # pools' bank-reuse WAR does not wait on them
                    mv_t2 = cvtile("mv_ps2")
                    mv_ps2 = mv_t2.rearrange("p a b -> p (a b)")
                    for dt in range(CT):
                        for ct in range(CT):
                            nc.tensor.matmul(mv_ps2[:, dt:dt + 1],
                                             wv8[:, ct // 2, ct % 2,
                                                 dt * 128:(dt + 1) * 128],
                                             shift8[:, ct, :],
                                             start=(ct == 0),
                                             stop=(ct == CT - 1))
                    nc.vector.tensor_tensor(v0col, mv_ps2[:, 0:CT],
                                            cols["bv"], ADD)
                    nc.vector.tensor_scalar_mul(v08[:, :, 0], v0col, 1.0)
                    mv_t3 = cvtile("mv_ps3")
                    mv_ps3 = mv_t3.rearrange("p a b -> p (a b)")
                    for dt in range(CT):
                        for ct in range(CT):
                            nc.tensor.matmul(mv_ps3[:, dt:dt + 1],
                                             w8["wo8"][:, ct // 2, dt,
                                                       ct % 2, :],
                                             v08[:, ct, :],
                                             start=(ct == 0),
                                             stop=(ct == CT - 1))
                    nc.vector.tensor_tensor(fbias, mv_ps3[:, 0:CT],
                                            cols["bo"], ADD)
                    for ct in range(CT):
                        nc.gpsimd.tensor_scalar(
                            x_t[:, ct, 0:NQ], x_t[:, ct, 0:NQ],
                            fbias[:, ct:ct + 1], None, ADD)

                for mc in range(MC):
                    if mc == 4:
                        fbias_matvecs()
                    for i in range(2):          # dt pair (2i, 2i+1)
                        kt = cvtile("kc").rearrange("p a b -> p (a b)") \
                            .rearrange("p (d mt m) -> p d mt m", d=2, m=128)
                        for d in range(2):
                            dt = 2 * i + d
                            for j in range(2):
                                nc.tensor.matmul(kt[:, d], wk8s[:, j, dt],
                                                 x8m(j, 4 * mc, 4),
                                                 start=(j == 0), stop=(j == 1),
                                                 perf_mode=DR)
                        if mc < MC - 3:
                            k_epi(kt, mc, i)()
                        else:
                            deferred_epis.append(k_epi(kt, mc, i))
                    for h in (2 * mc, 2 * mc + 1):  # msub pair (2h, 2h+1)
                        vt = cvtile("vc").rearrange("p a b -> p (a b)") \
                            .rearrange("p (g ct m) -> p g ct m", g=2, m=128)
                        for gi in range(2):
                            g = 2 * h + gi
                            for j in range(2):
                                nc.tensor.matmul(vt[:, gi], x8v[:, j, g],
                                                 wv8s[:, j],
                                                 start=(j == 0), stop=(j == 1),
                                                 perf_mode=DR)
                        if mc < MC - 3:
                            v_epi(vt, h)()
                        else:
                            deferred_epis.append(v_epi(vt, h))
                for t in range(1, CT):
                    q_conv(t)
                # preload the Exp activation table: this dummy exp DEPENDS on
                # the last q8 epilogue, pinning the (implicit) table load to
                # the conv tail where Act idles — an undepended dummy gets
                # scheduled early and steals Act time from conv epilogues
                nc.scalar.activation(out=warm, in_=q8[0:1, 1, NC - 1, 1, 0:1],
                                     func=EXP)


            # ---------------- attention (s x2 + pv + den/proj = 8 banks) ----
            with (
                tc.tile_pool(name="ps_s", bufs=2, space="PSUM") as ps_s,
                tc.tile_pool(name="ps_pv", bufs=1, space="PSUM") as ps_pv,
                tc.tile_pool(name="ps_o", bufs=1, space="PSUM") as ps_o,
            ):
                pending = None  # (nci, ao8)

                def tail_step(dts, ot, pr):
                    pnci, pao8 = pending
                    for dt in dts:
                        pr_ps = pr[:, dt % 2, :]
                        for j in range(2):
                            nc.tensor.matmul(pr_ps, w8["wo8"][:, j, dt],
                                             pao8[:, 2 * j:2 * j + 2, :],
                                             start=(j == 0), stop=(j == 1),
                                             perf_mode=DR,
                                             skip_group_check=True)
                        nc.vector.tensor_tensor(
                            ot[:, dt, :], pr_ps,
                            x_t[:, dt, pnci * 256:(pnci + 1) * 256], ADD)

                def tail_flush(ot):
                    pnci = pending[0]
                    nc.sync.dma_start(
                        out=rout[:, :, pnci * 256:(pnci + 1) * 256], in_=ot)

                def emit_pv(pm, pp, pv, den_ps):
                    # den first: its stop gates the reciprocal, so retiring
                    # it at block start shortens the divide chain
                    for u in range(2):
                        nc.tensor.matmul(
                            den_ps, ones8, pp[:, 2 * u:2 * u + 2, :],
                            start=(pm == 0 and u == 0),
                            stop=(pm == MC - 1 and u == 1), perf_mode=DR,
                            skip_group_check=True)
                        for ct in range(CT):
                            nc.tensor.matmul(
                                pv[ct], vT8[:, 2 * pm + u, ct],
                                pp[:, 2 * u:2 * u + 2, :],
                                start=(pm == 0 and u == 0),
                                stop=(pm == MC - 1 and u == 1),
                                perf_mode=DR, skip_group_check=True)

                def divide(pnci, prev_pv_all, den_ps):
                    # reciprocal + two half-divides (each half releases its
                    # pv regions and unblocks the matching proj j-step)
                    rec = statp.tile([128, 256], F32, tag="rec", name="rec")
                    nc.vector.reciprocal(rec, den_ps)
                    ao8 = ao8p.tile([128, CT, 256], FP8, tag="ao8", name="ao8")
                    rec_h = bass.AP(rec.tensor, rec.offset,
                                    [rec.ap[0], [0, 2], rec.ap[1]])
                    for hh in range(2):
                        nc.vector.tensor_tensor(
                            ao8[:, 2 * hh:2 * hh + 2, :],
                            prev_pv_all[:, 2 * hh:2 * hh + 2, :], rec_h, MULT)
                    return (pnci, ao8)

                prev = None  # (p_tiles, pv_all, den_ps) of nci-1
                prev_defer = None
                for nci in range(NC):
                    # defer-3 spreads each chunk's trailing PV over three
                    # steps of the next chunk; the last chunk stays defer-2
                    # so the drain keeps only two trailing blocks
                    defer = 2 if nci == NC - 1 else 3
                    p_tiles = []
                    pv_all = den_ps = pv = None
                    ot = otp.tile([128, CT, 256], F32, tag="ot", name="ot") \
                        if prev is not None else None
                    pr = ps_o.tile([128, 2, 256], F32, tag="o", name="pr") \
                        if prev is not None else None
                    for mc in range(MC):
                        s_ps = ps_s.tile([128, 4, 256], F32, tag="s", name="s_ps")
                        for msub in range(4):
                            for j in range(2):
                                nc.tensor.matmul(
                                    s_ps[:, msub, :], k8[:, j, 4 * mc + msub],
                                    q8[:, j, nci],
                                    start=(j == 0), stop=(j == 1), perf_mode=DR)
                        p8 = p8p.tile([128, 4, 256], FP8, tag="p8", name="p8")
                        p_tiles.append(p8)
                        nc.scalar.activation(out=p8, in_=s_ps, func=EXP)
                        if nci == 0 and mc >= 2 and deferred_epis:
                            deferred_epis.pop(0)()
                            if deferred_epis:
                                deferred_epis.pop(0)()
                        if prev is not None and mc < prev_defer:
                            # previous chunk's trailing PV + softmax divide
                            pp_, pva_, den_ = prev
                            emit_pv(MC - prev_defer + mc,
                                    pp_[MC - prev_defer + mc],
                                    [pva_[:, ct, :] for ct in range(CT)], den_)
                            if mc == prev_defer - 1:
                                pending = divide(nci - 1, pva_, den_)
                        if mc == defer:
                            # allocate AFTER the previous generation's readers
                            # (trailing PV + divide) are emitted, so the pool
                            # WAR edges cover them
                            pv_all = ps_pv.tile([128, 4, 256], F32, tag="pva",
                                                name="pva")
                            pv = [pv_all[:, ct, :] for ct in range(CT)]
                            den_ps = ps_pv.tile([128, 256], F32, tag="den",
                                                name="den")
                        if mc >= defer:
                            emit_pv(mc - defer, p_tiles[mc - defer], pv, den_ps)
                        if pending is not None and mc >= 4:
                            tail_step([mc - 4], ot, pr)
                            if mc == MC - 1:
                                tail_flush(ot)
                                pending = None
                    prev = (p_tiles, pv_all, den_ps)
                    prev_defer = defer
                # drain: last chunk's trailing PV, divide, proj, store
                pp_, pva_, den_ = prev
                pvl = [pva_[:, ct, :] for ct in range(CT)]
                emit_pv(MC - 2, pp_[MC - 2], pvl, den_)
                emit_pv(MC - 1, pp_[MC - 1], pvl, den_)
                # final drain pipelined by query-halves: divide, proj,
                # residual and store for queries 0:128 flow while 128:256 is
                # still dividing. Per-dt j-pair order and per-region
                # accumulation order are unchanged.
                pnci = NC - 1
                rec = statp.tile([128, 256], F32, tag="rec", name="rec_f")
                nc.vector.reciprocal(rec, den_)
                ao8 = ao8p.tile([128, CT, 256], FP8, tag="ao8", name="ao8_f")
                ot = otp.tile([128, CT, 256], F32, tag="ot", name="ot_f")
                for qh in range(2):
                    qs = slice(128 * qh, 128 * qh + 128)
                    rec_q = bass.AP(rec.tensor, rec.offset + 128 * qh,
                                    [rec.ap[0], [0, CT], [1, 128]])
                    nc.vector.tensor_tensor(ao8[:, :, qs], pva_[:, :, qs],
                                            rec_q, MULT)
                for qh in range(2):
                    qs = slice(128 * qh, 128 * qh + 128)
                    for dt in range(CT):
                        pr_ps = pva_[:, dt, qs]
                        for j in range(2):
                            nc.tensor.matmul(pr_ps, w8["wo8"][:, j, dt],
                                             ao8[:, 2 * j:2 * j + 2, qs],
                                             start=(j == 0), stop=(j == 1),
                                             perf_mode=DR,
                                             skip_group_check=True)
                    nc.vector.tensor_tensor(
                        ot[:, :, qs], pva_[:, :, qs],
                        x_t[:, :, pnci * 256 + 128 * qh:
                            pnci * 256 + 128 * qh + 128], ADD)
                    nc.sync.dma_start(
                        out=rout[:, :, pnci * 256 + 128 * qh:
                                 pnci * 256 + 128 * qh + 128],
                        in_=ot[:, :, qs])
    nc.finalize()
    return nc


_NC_CACHE = {}


def _get_nc():
    if "nc" not in _NC_CACHE:
        _NC_CACHE["nc"] = _build()
    return _NC_CACHE["nc"]


def _prep_stationary(w):
    # w: [cout, cin] conv weight -> stationary DR layout [p, j, dt, ctp, m]
    wT = np.ascontiguousarray(w.T)                      # [cin, cout]
    arr = wT.reshape(2, 2, 128, 4, 128)                  # [j, ctp, p, dt, m]
    arr = np.transpose(arr, (2, 0, 3, 1, 4))             # [p, j, dt, ctp, m]
    return np.ascontiguousarray(arr).astype(ml_dtypes.float8_e4m3)


def _prep_moving(w):
    # w: [cout, cin] -> moving DR layout [p, j, ctp, cout]
    wT = np.ascontiguousarray(w.T)                      # [cin, cout]
    arr = wT.reshape(2, 2, 128, 512)                     # [j, ctp, p, cout]
    arr = np.transpose(arr, (2, 0, 1, 3))                # [p, j, ctp, cout]
    return np.ascontiguousarray(arr).astype(ml_dtypes.float8_e4m3)


def kernel(**inputs):
    x = np.ascontiguousarray(np.asarray(inputs["x"], dtype=np.float32))
    gamma = np.asarray(inputs["gamma"], np.float32)
    beta = np.asarray(inputs["beta"], np.float32)
    w = {n: np.asarray(inputs[n], np.float32) for n in ("wq", "wk", "wv", "wo")}
    b = {n: np.asarray(inputs[n], np.float32) for n in ("bq", "bk", "bv", "bo")}

    mg_np = np.zeros((C, G), np.float32)
    mg_np[np.arange(C), np.arange(C) // GS] = 1.0 / GS
    common = {
        "Mg": mg_np,
        "M2": np.ascontiguousarray((mg_np != 0).astype(np.float32).T),
        "wq8": _prep_stationary(w["wq"]),
        "wk8": _prep_stationary(w["wk"]),
        "wo8": _prep_stationary(w["wo"]),
        "wv8m": _prep_moving(w["wv"]),
        "vec5": np.ascontiguousarray(
            np.stack([gamma, beta, b["bq"], b["bv"], b["bo"]])),
    }
    in_maps = []
    for core in range(N_CORES):
        bi, ch = divmod(core, 2)
        xi = x[bi].reshape(C, HW)
        if ch:
            xi = np.roll(xi, -NQ, axis=1)
        m = dict(common)
        m["x_img"] = np.ascontiguousarray(xi)
        in_maps.append(m)

    want_trace = bool(int(os.environ.get("KTRACE", "0")))
    if not want_trace:
        os.environ["BASS_NEVER_TRACE"] = "1"
    global LAST_RESULTS
    LAST_RESULTS = run_bass_kernel_spmd(
        _get_nc(), in_maps, core_ids=list(range(N_CORES)), trace=want_trace)
    full = np.empty((B, C, HW), np.float32)
    for core in range(N_CORES):
        bi, ch = divmod(core, 2)
        full[bi][:, ch * NQ:(ch + 1) * NQ] = LAST_RESULTS.results[core]["out"]
    return full.reshape(B, C, H, W)


# revision 48
# speedup vs baseline: 1.0038x; 1.0014x over previous
"""AttnBlock for Trainium2, 8 NeuronCores — fp8e4 DoubleRow rewrite (v9).

Sharding: core i = (batch i//2, query-half i%2). Full K/V per core, no
collectives. One program for all cores: odd cores get the image columns
rolled by 2048 (attention is permutation-equivariant over key positions;
GroupNorm stats are order-invariant), so every core computes queries 0..2047
of its (possibly rolled) image.

Math (exact rearrangement of the reference):
  GroupNorm h = scale*x + shift; scale is folded into the fp8 conv WEIGHTS
  on device (w' = fp8(w8 * scale_cin)), so the x->fp8 cast needs no scale
  and runs chunk-by-chunk behind the input DMA, before stats complete.
  shift contributions: k-conv -> constant along m, dropped with bk (softmax
  invariant); v-conv -> v0 = Wv shift + bv contributes Wo v0 to every output
  (softmax rows sum to 1) -> fbias = Wo v0 + bo, pre-added into x in place;
  q-conv -> q0 = Wq shift + bq added in the q epilogue. 1/sqrt(C) is applied
  in the k epilogue. Mg carries 1/GS from the host so group sums emerge as
  [mean, E[x^2]] directly.

Pipeline (cost-model driven; all numbers per core):
  - Phase 1 (~27us, DMA-bound): x streams in 512-col chunks; bn_stats on
    DVE; ONE fp8 copy of x is cast in v-conv stationary layout
    [128, j, mt, ctp, 128] (Act 3/4, Pool 1/4). The k/q convs read their
    MOVING operand from the same tile through a strided view, so the second
    fp8 copy of the old design is gone. A dummy Sqrt at t~0 keeps the
    activation-table load off the stats chain.
  - Convs (~24us): [128,4,256] PSUM ring (4 bufs = all 8 banks), whole-tile
    [128,1024] PSUM->fp8 epilogues alternating Act/DVE (GPSIMD has no PSUM
    port). q convs for the first n-chunk lead; the last three m-chunks'
    k/v epilogues are deferred into attention where the engines have slack.
    The v0/fbias matvecs run mid-conv (off the stats head chain AND off
    the ring tail that the attention pools' bank-reuse waits on).
  - Attention (~73us, jointly exp/PE-bound): per m-chunk, 8 score matmuls
    -> ONE exp(P) [128,4,256] on Act -> PV+den DoubleRow matmuls deferred
    THREE chunks (two for the final chunk, keeping the drain short) so the
    exp engine+sem latency never blocks the PE and the trailing blocks
    spread evenly over the next chunk's first steps. The four
    PV accumulators share 2 PSUM banks as concurrently-open accumulation
    groups (HW start_tensor_calc zeroes only the written region; the sim's
    one-group-per-bank check is skipped). Each chunk's trailing PV pair,
    reciprocal, and softmax divide (one stride-0-broadcast DVE op per half)
    are carried into the next chunk's first steps; proj+residual tails for
    chunk i-1 run at steps 3/5/7 of chunk i. PSUM: s x2 (4 banks) + packed
    PV (2) + den (1) + proj scratch (1).
  - Drain: the freed PV regions double as four independent proj scratches
    (no write-after-read ladder); one wide residual-add and one store.

Known-good invariants learned the hard way:
  - DoubleRow STATIONARY needs its row-pair blocks contiguous in SBUF;
    moving operands tolerate arbitrary strides.
  - Concurrent PSUM accumulation groups in one bank work when they start
    together (PV packing), but a transient group start/stopping while
    another group is mid-accumulation in the same bank corrupts it (the
    den+proj bank-sharing experiment).
  - Pool tiles capture their WAR readers at allocation time: allocate a
    pool generation only after every reader of the previous generation has
    been emitted (the carried-divide race).
"""

import math
import os
import sys

sys.path.insert(0, "/opt/trn_rl_repo")

import numpy as np
import ml_dtypes

import concourse.bacc as bacc
import concourse.bass as bass
import concourse.mybir as mybir
import concourse.tile as tile
from concourse.bass_utils import run_bass_kernel_spmd

F32 = mybir.dt.float32
F32R = mybir.dt.float32r
FP8 = mybir.dt.float8e4
DR = mybir.MatmulPerfMode.DoubleRow
MULT = mybir.AluOpType.mult
ADD = mybir.AluOpType.add
SUB = mybir.AluOpType.subtract
EXP = mybir.ActivationFunctionType.Exp
IDENT = mybir.ActivationFunctionType.Identity
COPY = mybir.ActivationFunctionType.Copy
SQRT = mybir.ActivationFunctionType.Sqrt

B, C, H, W = 4, 512, 64, 64
HW = H * W
G = 32
GS = C // G
NQ = HW // 2
EPS = 1e-5
N_CORES = 8
CT = C // 128
MC = HW // 512          # 8 m-chunks
NC = NQ // 256          # 8 n-chunks
INV_SQRT_C = 1.0 / math.sqrt(C)

LAST_RESULTS = None


def _build():
    nc = bacc.Bacc("TRN2", target_bir_lowering=False, debug=False)

    x_d = nc.dram_tensor("x_img", [C, HW], F32R, kind="ExternalInput").ap()
    w_st = {n: nc.dram_tensor(n, [128, 2, 4, 2, 128], FP8, kind="ExternalInput").ap()
            for n in ("wq8", "wk8", "wo8")}
    wv8m_d = nc.dram_tensor("wv8m", [128, 2, 2, 512], FP8, kind="ExternalInput").ap()
    mg_d = nc.dram_tensor("Mg", [C, G], F32, kind="ExternalInput").ap()
    m2_d = nc.dram_tensor("M2", [G, C], F32, kind="ExternalInput").ap()
    # gamma, beta, bq, bv, bo packed as one [5, C] tensor -> [128, 5, CT] cols
    v5_d = nc.dram_tensor("vec5", [5, C], F32, kind="ExternalInput").ap()
    out_d = nc.dram_tensor("out", [C, NQ], F32, kind="ExternalOutput").ap()

    rx = x_d.rearrange("(t p) m -> p t m", p=128)
    rout = out_d.rearrange("(t p) n -> p t n", p=128)

    with tile.TileContext(nc) as tc:
        with (
            tc.tile_pool(name="singles", bufs=1) as singles,
            tc.tile_pool(name="statp", bufs=3) as statp,
            tc.tile_pool(name="p8p", bufs=6) as p8p,
            tc.tile_pool(name="ao8p", bufs=3) as ao8p,
            tc.tile_pool(name="otp", bufs=3) as otp,
        ):
            # ------ pass 1: x DMA first (the big transfer paces everything);
            # per chunk: bn_stats (DVE only) + x8 cast (Act 3/4, Pool 1/4)
            x_t = singles.tile([128, CT, HW], F32R, tag="x_t")
            stats_all = singles.tile([128, CT, MC + 1, 6], F32, tag="stats_all")
            # Single fp8 copy of x in v-conv stationary layout
            # [128, j, mt, ctp, 128] (ctp pair blocks contiguous, as DoubleRow
            # stationary requires). The k/q convs read their MOVING operand
            # from the same tile through a strided [128, ctp, mt, 128] view.
            x8v = singles.tile([128, 2, HW // 128, 2, 128], FP8, tag="x8v")
            for mc in range(MC):
                ms = slice(mc * 512, (mc + 1) * 512)
                if mc in (0, MC - 1):
                    # split the first chunk (descriptor-generation latency
                    # shouldn't delay first bytes) and the last chunk (its
                    # first-half bn_stats overlap the second half's DMA,
                    # shortening the stats tail; bn_stats are sufficient
                    # statistics so unequal segments aggregate exactly)
                    h0 = slice(mc * 512, mc * 512 + 256)
                    h1 = slice(mc * 512 + 256, (mc + 1) * 512)
                    nc.sync.dma_start(out=x_t[:, :, h0], in_=rx[:, :, h0])
                    nc.sync.dma_start(out=x_t[:, :, h1], in_=rx[:, :, h1])
                else:
                    nc.sync.dma_start(out=x_t[:, :, ms], in_=rx[:, :, ms])
                if mc == MC - 1:
                    h0 = slice(mc * 512, mc * 512 + 256)
                    h1 = slice(mc * 512 + 256, (mc + 1) * 512)
                    for t in range(CT):
                        nc.vector.bn_stats(out=stats_all[:, t, mc, :],
                                           in_=x_t[:, t, h0])
                        nc.vector.bn_stats(out=stats_all[:, t, MC, :],
                                           in_=x_t[:, t, h1])
                else:
                    for t in range(CT):
                        nc.vector.bn_stats(out=stats_all[:, t, mc, :],
                                           in_=x_t[:, t, ms])
                if mc % 2 == 1:
                    m2s = slice((mc - 1) * 512, (mc + 1) * 512)
                    for t in range(CT):
                        dst = x8v[:, t // 2, 4 * mc - 4:4 * mc + 4, t % 2, :]
                        src = x_t[:, t, m2s].rearrange("p (mt m) -> p mt m",
                                                       m=128)
                        if t < 3:
                            nc.scalar.activation(out=dst, in_=src, func=COPY)
                        else:
                            nc.gpsimd.tensor_copy(out=dst, in_=src)

            # ---------------- constants / small loads ----------------
            v5 = singles.tile([128, 5, CT], F32, tag="v5")
            nc.sync.dma_start(out=v5, in_=v5_d.rearrange("v (t p) -> p v t", p=128))
            cols = {n: v5[:, i, :]
                    for i, n in enumerate(("gamma", "beta", "bq", "bv", "bo"))}
            eps_t = singles.tile([G, 1], F32, tag="eps")
            nc.vector.memset(eps_t, EPS)
            # touch Sqrt once at t~0 so its activation table is resident
            # before the GroupNorm stats chain needs it
            warm = singles.tile([1, 1], F32, tag="warm")
            nc.vector.memset(warm, 1.0)
            nc.scalar.activation(out=warm, in_=warm, func=SQRT, bias=0.0,
                                 scale=1.0)
            ones8 = singles.tile([128, 2, 128], FP8, tag="ones8")
            nc.vector.memset(ones8, 1.0)
            Mg = singles.tile([128, CT, G], F32, tag="Mg")
            nc.sync.dma_start(out=Mg, in_=mg_d.rearrange("(t p) g -> p t g", p=128))
            M2 = singles.tile([G, CT, 128], F32, tag="M2")
            nc.sync.dma_start(out=M2, in_=m2_d.rearrange("g (t p) -> g t p", p=128))

            w8 = {}
            for n in ("wq8", "wk8", "wo8"):
                t = singles.tile([128, 2, 4, 2, 128], FP8, tag=n, name=n)
                nc.scalar.dma_start(out=t, in_=w_st[n])
                w8[n] = t
            wv8 = singles.tile([128, 2, 2, 512], FP8, tag="wv8m")
            nc.scalar.dma_start(out=wv8, in_=wv8m_d)

            def x8m(j, blk0, nblk):
                # moving view for k/q convs: [128, ctp, mt, 128]
                return x8v[:, j, blk0:blk0 + nblk, :, :].rearrange(
                    "p mt ctp m -> p ctp mt m")
            mv = statp.tile([128, CT, 2], F32, tag="mv")
            for t in range(CT):
                nc.vector.bn_aggr(out=mv[:, t, :], in_=stats_all[:, t, :, :])
            s_cat = statp.tile([128, CT, 2], F32, tag="s_cat")
            nc.vector.tensor_copy(out=s_cat[:, :, 0:1], in_=mv[:, :, 0:1])
            nc.vector.tensor_tensor(s_cat[:, :, 1:2], mv[:, :, 0:1], mv[:, :, 0:1], MULT)
            nc.vector.tensor_tensor(s_cat[:, :, 1:2], s_cat[:, :, 1:2], mv[:, :, 1:2], ADD)

            k8 = singles.tile([128, 2, HW // 128, 2, 128], FP8, tag="k8")
            vT8 = singles.tile([128, HW // 256, CT, 2, 128], FP8, tag="vT8")
            q8 = singles.tile([128, 2, NC, 2, 256], FP8, tag="q8")

            # ---------------- conv phase: dedicated 6-bank ring ----------------
            with tc.tile_pool(name="ps_cv", bufs=4, space="PSUM") as ps_cv:
                def cvtile(name):
                    return ps_cv.tile([128, 4, 256], F32, tag="cv", name=name)

                # ---- GroupNorm stats -> per-channel scale/shift ----
                # (Mg carries 1/GS from the host, so gsum = [mean_g, E2_g])
                gsum_t = cvtile("gsum")
                gsum_ps = gsum_t.rearrange("p a b -> p (a b)")
                for ct in range(CT):
                    nc.tensor.matmul(gsum_ps[0:G, 0:2], Mg[:, ct, :], s_cat[:, ct, :],
                                     start=(ct == 0), stop=(ct == CT - 1))
                gme = statp.tile([G, 2], F32, tag="gme")
                nc.vector.tensor_copy(out=gme, in_=gsum_ps[0:G, 0:2])
                gvar = statp.tile([G, 1], F32, tag="gvar")
                nc.vector.tensor_tensor(gvar, gme[:, 0:1], gme[:, 0:1], MULT)
                nc.vector.tensor_tensor(gvar, gme[:, 1:2], gvar, SUB)
                grstd = statp.tile([G, 2], F32, tag="grstd")
                nc.scalar.activation(out=gvar, in_=gvar, func=SQRT, bias=eps_t, scale=1.0)
                nc.vector.reciprocal(grstd[:, 0:1], gvar)
                nc.vector.tensor_copy(out=grstd[:, 1:2], in_=gme[:, 0:1])
                # rm: all four [rstd, mean] channel broadcasts in one tile
                rm_pc = statp.tile([128, CT, 2], F32, tag="rm_pc")
                rm_t = cvtile("rm")
                rm_ps = rm_t.rearrange("p a b -> p (a b)")
                for ct in range(CT):
                    nc.tensor.matmul(rm_ps[:, 2 * ct:2 * ct + 2], M2[:, ct, :],
                                     grstd, start=True, stop=True)
                nc.vector.tensor_copy(
                    out=rm_pc,
                    in_=rm_ps[:, 0:2 * CT].rearrange("p (ct two) -> p ct two",
                                                     two=2))
                scale_pc = singles.tile([128, CT], F32, tag="scale_pc")
                shift_pc = singles.tile([128, CT], F32, tag="shift_pc")
                nc.vector.tensor_tensor(scale_pc, cols["gamma"], rm_pc[:, :, 0], MULT)

                # ---- scaled weight copies on Act+DVE (originals stay for
                #      the matvecs, so no WAR chain). q and k first: their
                #      convs lead the PE stream.
                wk8s = singles.tile([128, 2, 4, 2, 128], FP8, tag="wk8s")
                wv8s = singles.tile([128, 2, 2, 512], FP8, tag="wv8s")
                wq8s = singles.tile([128, 2, 4, 2, 128], FP8, tag="wq8s")

                def wscale(dst, src, ct):
                    sc = scale_pc[:, ct:ct + 1]
                    if ct % 2 == 0:
                        nc.scalar.activation(out=dst, in_=src, func=COPY, scale=sc)
                    else:
                        nc.vector.tensor_scalar(dst, src, sc, None, MULT)

                for ct in range(CT):
                    j, p = ct // 2, ct % 2
                    wscale(wq8s[:, j, :, p, :], w8["wq8"][:, j, :, p, :], ct)
                for ct in range(CT):
                    j, p = ct // 2, ct % 2
                    wscale(wk8s[:, j, :, p, :], w8["wk8"][:, j, :, p, :], ct)
                for ct in range(CT):
                    j, p = ct // 2, ct % 2
                    nc.gpsimd.tensor_scalar(wv8s[:, j, p, :], wv8[:, j, p, :],
                                            scale_pc[:, ct:ct + 1], None, MULT)

                nc.vector.tensor_tensor(shift_pc, scale_pc, rm_pc[:, :, 1], MULT)
                nc.vector.tensor_tensor(shift_pc, cols["beta"], shift_pc, SUB)
                shift8 = singles.tile([128, CT, 1], FP8, tag="shift8")
                nc.vector.tensor_scalar_mul(shift8[:, :, 0], shift_pc, 1.0)

                # ---- matvecs on UNscaled weights: q0, v0, fbias ----
                q0col = singles.tile([128, CT], F32, tag="q0col")
                v0col = singles.tile([128, CT], F32, tag="v0col")
                v08 = singles.tile([128, CT, 1], FP8, tag="v08")
                fbias = singles.tile([128, CT], F32, tag="fbias")
                mv_t = cvtile("mv_ps")
                mv_ps = mv_t.rearrange("p a b -> p (a b)")
                for dt in range(CT):
                    for ct in range(CT):
                        nc.tensor.matmul(mv_ps[:, dt:dt + 1],
                                         w8["wq8"][:, ct // 2, dt, ct % 2, :],
                                         shift8[:, ct, :],
                                         start=(ct == 0), stop=(ct == CT - 1))
                nc.vector.tensor_tensor(q0col, mv_ps[:, 0:CT], cols["bq"], ADD)
                # ---- convs; whole-tile [128,1024] epilogues alternate
                #      Act/DVE. q for t=0 first (attention needs q8[0]);
                #      the rest of q after k/v (consumed much later).
                def q_conv(t):
                    for i in range(2):
                        qt = cvtile("qc").rearrange("p a b -> p (a b)") \
                            .rearrange("p (d two n) -> p d two n", d=2, n=256)
                        for d in range(2):
                            dt = 2 * i + d
                            for j in range(2):
                                nc.tensor.matmul(qt[:, d], wq8s[:, j, dt],
                                                 x8m(j, 4 * t, 4),
                                                 start=(j == 0), stop=(j == 1),
                                                 perf_mode=DR)
                        for d in range(2):
                            dt = 2 * i + d
                            dst = q8[:, dt // 2, 2 * t:2 * t + 2, dt % 2, :]
                            if d == 0:
                                nc.scalar.activation(
                                    out=dst, in_=qt[:, d], func=IDENT,
                                    bias=q0col[:, dt:dt + 1], scale=1.0)
                            else:
                                nc.vector.tensor_scalar(
                                    dst, qt[:, d], q0col[:, dt:dt + 1], None, ADD)

                q_conv(0)
                deferred_epis = []

                def k_epi(kt, mc, i):
                    def run():
                        dst = k8[:, i, 4 * mc:4 * mc + 4, :, :]
                        src = kt.rearrange("p d mt m -> p mt d m")
                        if i == 0:
                            nc.scalar.activation(out=dst, in_=src,
                                                 func=COPY, scale=INV_SQRT_C)
                        else:
                            nc.vector.tensor_scalar(dst, src,
                                                    INV_SQRT_C, None, MULT)
                    return run

                def v_epi(vt, h):
                    def run():
                        dst = vT8[:, h, :, :, :]
                        src = vt.rearrange("p g ct m -> p ct g m")
                        if (h % 2 == 1 and h != 15) or h in (6, 14):
                            nc.scalar.activation(out=dst, in_=src, func=COPY)
                        else:
                            nc.vector.tensor_copy(out=dst, in_=src)
                    return run

                def fbias_matvecs():
                    # v0/fbias matvecs mid-conv: late enough to stay off the
                    # stats->wk8s head chain, early enough that the attention
                    # pools' bank-reuse WAR does not wait on them
                    mv_t2 = cvtile("mv_ps2")
                    mv_ps2 = mv_t2.rearrange("p a b -> p (a b)")
                    for dt in range(CT):
                        for ct in range(CT):
                            nc.tensor.matmul(mv_ps2[:, dt:dt + 1],
                                             wv8[:, ct // 2, ct % 2,
                                                 dt * 128:(dt + 1) * 128],
                                             shift8[:, ct, :],
                                             start=(ct == 0),
                                             stop=(ct == CT - 1))
                    nc.vector.tensor_tensor(v0col, mv_ps2[:, 0:CT],
                                            cols["bv"], ADD)
                    nc.vector.tensor_scalar_mul(v08[:, :, 0], v0col, 1.0)
                    mv_t3 = cvtile("mv_ps3")
                    mv_ps3 = mv_t3.rearrange("p a b -> p (a b)")
                    for dt in range(CT):
                        for ct in range(CT):
                            nc.tensor.matmul(mv_ps3[:, dt:dt + 1],
                                             w8["wo8"][:, ct // 2, dt,
                                                       ct % 2, :],
                                             v08[:, ct, :],
                                             start=(ct == 0),
                                             stop=(ct == CT - 1))
                    nc.vector.tensor_tensor(fbias, mv_ps3[:, 0:CT],
                                            cols["bo"], ADD)
                    for ct in range(CT):
                        nc.gpsimd.tensor_scalar(
                            x_t[:, ct, 0:NQ], x_t[:, ct, 0:NQ],
                            fbias[:, ct:ct + 1], None, ADD)

                for mc in range(MC):
                    if mc == 4:
                        fbias_matvecs()
                    for i in range(2):          # dt pair (2i, 2i+1)
                        kt = cvtile("kc").rearrange("p a b -> p (a b)") \
                            .rearrange("p (d mt m) -> p d mt m", d=2, m=128)
                        for d in range(2):
                            dt = 2 * i + d
                            for j in range(2):
                                nc.tensor.matmul(kt[:, d], wk8s[:, j, dt],
                                                 x8m(j, 4 * mc, 4),
                                                 start=(j == 0), stop=(j == 1),
                                                 perf_mode=DR)
                        if mc < MC - 3:
                            k_epi(kt, mc, i)()
                        else:
                            deferred_epis.append(k_epi(kt, mc, i))
                    for h in (2 * mc, 2 * mc + 1):  # msub pair (2h, 2h+1)
                        vt = cvtile("vc").rearrange("p a b -> p (a b)") \
                            .rearrange("p (g ct m) -> p g ct m", g=2, m=128)
                        for gi in range(2):
                            g = 2 * h + gi
                            for j in range(2):
                                nc.tensor.matmul(vt[:, gi], x8v[:, j, g],
                                                 wv8s[:, j],
                                                 start=(j == 0), stop=(j == 1),
                                                 perf_mode=DR)
                        if mc < MC - 3:
                            v_epi(vt, h)()
                        else:
                            deferred_epis.append(v_epi(vt, h))
                for t in range(1, CT):
                    q_conv(t)
                # preload the Exp activation table: this dummy exp DEPENDS on
                # the last q8 epilogue, pinning the (implicit) table load to
                # the conv tail where Act idles — an undepended dummy gets
                # scheduled early and steals Act time from conv epilogues
                nc.scalar.activation(out=warm, in_=q8[0:1, 1, NC - 1, 1, 0:1],
                                     func=EXP)


            # ---------------- attention (s x2 + pv + den/proj = 8 banks) ----
            with (
                tc.tile_pool(name="ps_s", bufs=2, space="PSUM") as ps_s,
                tc.tile_pool(name="ps_pv", bufs=1, space="PSUM") as ps_pv,
                tc.tile_pool(name="ps_o", bufs=1, space="PSUM") as ps_o,
            ):
                pending = None  # (nci, ao8)

                def tail_step(dts, ot, pr):
                    pnci, pao8 = pending
                    for dt in dts:
                        pr_ps = pr[:, dt % 2, :]
                        for j in range(2):
                            nc.tensor.matmul(pr_ps, w8["wo8"][:, j, dt],
                                             pao8[:, 2 * j:2 * j + 2, :],
                                             start=(j == 0), stop=(j == 1),
                                             perf_mode=DR,
                                             skip_group_check=True)
                        nc.vector.tensor_tensor(
                            ot[:, dt, :], pr_ps,
                            x_t[:, dt, pnci * 256:(pnci + 1) * 256], ADD)

                def tail_flush(ot):
                    pnci = pending[0]
                    nc.sync.dma_start(
                        out=rout[:, :, pnci * 256:(pnci + 1) * 256], in_=ot)

                def emit_pv(pm, pp, pv, den_ps):
                    # den first: its stop gates the reciprocal, so retiring
                    # it at block start shortens the divide chain
                    for u in range(2):
                        nc.tensor.matmul(
                            den_ps, ones8, pp[:, 2 * u:2 * u + 2, :],
                            start=(pm == 0 and u == 0),
                            stop=(pm == MC - 1 and u == 1), perf_mode=DR,
                            skip_group_check=True)
                        for ct in range(CT):
                            nc.tensor.matmul(
                                pv[ct], vT8[:, 2 * pm + u, ct],
                                pp[:, 2 * u:2 * u + 2, :],
                                start=(pm == 0 and u == 0),
                                stop=(pm == MC - 1 and u == 1),
                                perf_mode=DR, skip_group_check=True)

                def divide(pnci, prev_pv_all, den_ps):
                    # reciprocal + two half-divides (each half releases its
                    # pv regions and unblocks the matching proj j-step)
                    rec = statp.tile([128, 256], F32, tag="rec", name="rec")
                    nc.vector.reciprocal(rec, den_ps)
                    ao8 = ao8p.tile([128, CT, 256], FP8, tag="ao8", name="ao8")
                    rec_h = bass.AP(rec.tensor, rec.offset,
                                    [rec.ap[0], [0, 2], rec.ap[1]])
                    for hh in range(2):
                        nc.vector.tensor_tensor(
                            ao8[:, 2 * hh:2 * hh + 2, :],
                            prev_pv_all[:, 2 * hh:2 * hh + 2, :], rec_h, MULT)
                    return (pnci, ao8)

                prev = None  # (p_tiles, pv_all, den_ps) of nci-1
                prev_defer = None
                for nci in range(NC):
                    # defer-3 spreads each chunk's trailing PV over three
                    # steps of the next chunk; the last chunk stays defer-2
                    # so the drain keeps only two trailing blocks
                    defer = 2 if nci == NC - 1 else 3
                    p_tiles = []
                    pv_all = den_ps = pv = None
                    ot = otp.tile([128, CT, 256], F32, tag="ot", name="ot") \
                        if prev is not None else None
                    pr = ps_o.tile([128, 2, 256], F32, tag="o", name="pr") \
                        if prev is not None else None
                    for mc in range(MC):
                        s_ps = ps_s.tile([128, 4, 256], F32, tag="s", name="s_ps")
                        for msub in range(4):
                            for j in range(2):
                                nc.tensor.matmul(
                                    s_ps[:, msub, :], k8[:, j, 4 * mc + msub],
                                    q8[:, j, nci],
                                    start=(j == 0), stop=(j == 1), perf_mode=DR)
                        p8 = p8p.tile([128, 4, 256], FP8, tag="p8", name="p8")
                        p_tiles.append(p8)
                        nc.scalar.activation(out=p8, in_=s_ps, func=EXP)
                        if nci == 0 and mc >= 2 and deferred_epis:
                            deferred_epis.pop(0)()
                            if deferred_epis:
                                deferred_epis.pop(0)()
                        if prev is not None and mc < prev_defer:
                            # previous chunk's trailing PV + softmax divide
                            pp_, pva_, den_ = prev
                            emit_pv(MC - prev_defer + mc,
                                    pp_[MC - prev_defer + mc],
                                    [pva_[:, ct, :] for ct in range(CT)], den_)
                            if mc == prev_defer - 1:
                                pending = divide(nci - 1, pva_, den_)
                        if mc == defer:
                            # allocate AFTER the previous generation's readers
                            # (trailing PV + divide) are emitted, so the pool
                            # WAR edges cover them
                            pv_all = ps_pv.tile([128, 4, 256], F32, tag="pva",
                                                name="pva")
                            pv = [pv_all[:, ct, :] for ct in range(CT)]
                            den_ps = ps_pv.tile([128, 256], F32, tag="den",
                                                name="den")
                        if mc >= defer:
                            emit_pv(mc - defer, p_tiles[mc - defer], pv, den_ps)
                        if pending is not None and mc >= 4:
                            tail_step([mc - 4], ot, pr)
                            if mc == MC - 1:
                                tail_flush(ot)
                                pending = None
                    prev = (p_tiles, pv_all, den_ps)
                    prev_defer = defer
                # drain: last chunk's trailing PV, divide, proj, store
                pp_, pva_, den_ = prev
                pvl = [pva_[:, ct, :] for ct in range(CT)]
                emit_pv(MC - 2, pp_[MC - 2], pvl, den_)
                emit_pv(MC - 1, pp_[MC - 1], pvl, den_)
                # final drain pipelined by query-halves: divide, proj,
                # residual and store for queries 0:128 flow while 128:256 is
                # still dividing. Per-dt j-pair order and per-region
                # accumulation order are unchanged.
                pnci = NC - 1
                rec = statp.tile([128, 256], F32, tag="rec", name="rec_f")
                nc.vector.reciprocal(rec, den_)
                ao8 = ao8p.tile([128, CT, 256], FP8, tag="ao8", name="ao8_f")
                ot = otp.tile([128, CT, 256], F32, tag="ot", name="ot_f")
                for qh in range(2):
                    qs = slice(128 * qh, 128 * qh + 128)
                    rec_q = bass.AP(rec.tensor, rec.offset + 128 * qh,
                                    [rec.ap[0], [0, CT], [1, 128]])
                    nc.vector.tensor_tensor(ao8[:, :, qs], pva_[:, :, qs],
                                            rec_q, MULT)
                for qh in range(2):
                    qs = slice(128 * qh, 128 * qh + 128)
                    for dt in range(CT):
                        pr_ps = pva_[:, dt, qs]
                        for j in range(2):
                            nc.tensor.matmul(pr_ps, w8["wo8"][:, j, dt],
                                             ao8[:, 2 * j:2 * j + 2, qs],
                                             start=(j == 0), stop=(j == 1),
                                             perf_mode=DR,
                                             skip_group_check=True)
                    nc.vector.tensor_tensor(
                        ot[:, :, qs], pva_[:, :, qs],
                        x_t[:, :, pnci * 256 + 128 * qh:
                            pnci * 256 + 128 * qh + 128], ADD)
                    nc.sync.dma_start(
                        out=rout[:, :, pnci * 256 + 128 * qh:
                                 pnci * 256 + 128 * qh + 128],
                        in_=ot[:, :, qs])
    nc.finalize()
    return nc


_NC_CACHE = {}


def _get_nc():
    if "nc" not in _NC_CACHE:
        _NC_CACHE["nc"] = _build()
    return _NC_CACHE["nc"]


def _prep_stationary(w):
    # w: [cout, cin] conv weight -> stationary DR layout [p, j, dt, ctp, m]
    wT = np.ascontiguousarray(w.T)                      # [cin, cout]
    arr = wT.reshape(2, 2, 128, 4, 128)                  # [j, ctp, p, dt, m]
    arr = np.transpose(arr, (2, 0, 3, 1, 4))             # [p, j, dt, ctp, m]
    return np.ascontiguousarray(arr).astype(ml_dtypes.float8_e4m3)


def _prep_moving(w):
    # w: [cout, cin] -> moving DR layout [p, j, ctp, cout]
    wT = np.ascontiguousarray(w.T)                      # [cin, cout]
    arr = wT.reshape(2, 2, 128, 512)                     # [j, ctp, p, cout]
    arr = np.transpose(arr, (2, 0, 1, 3))                # [p, j, ctp, cout]
    return np.ascontiguousarray(arr).astype(ml_dtypes.float8_e4m3)


def kernel(**inputs):
    x = np.ascontiguousarray(np.asarray(inputs["x"], dtype=np.float32))
    gamma = np.asarray(inputs["gamma"], np.float32)
    beta = np.asarray(inputs["beta"], np.float32)
    w = {n: np.asarray(inputs[n], np.float32) for n in ("wq", "wk", "wv", "wo")}
    b = {n: np.asarray(inputs[n], np.float32) for n in ("bq", "bk", "bv", "bo")}

    mg_np = np.zeros((C, G), np.float32)
    mg_np[np.arange(C), np.arange(C) // GS] = 1.0 / GS
    common = {
        "Mg": mg_np,
        "M2": np.ascontiguousarray((mg_np != 0).astype(np.float32).T),
        "wq8": _prep_stationary(w["wq"]),
        "wk8": _prep_stationary(w["wk"]),
        "wo8": _prep_stationary(w["wo"]),
        "wv8m": _prep_moving(w["wv"]),
        "vec5": np.ascontiguousarray(
            np.stack([gamma, beta, b["bq"], b["bv"], b["bo"]])),
    }
    in_maps = []
    for core in range(N_CORES):
        bi, ch = divmod(core, 2)
        xi = x[bi].reshape(C, HW)
        if ch:
            xi = np.roll(xi, -NQ, axis=1)
        m = dict(common)
        m["x_img"] = np.ascontiguousarray(xi)
        in_maps.append(m)

    want_trace = bool(int(os.environ.get("KTRACE", "0")))
    if not want_trace:
        os.environ["BASS_NEVER_TRACE"] = "1"
    global LAST_RESULTS
    LAST_RESULTS = run_bass_kernel_spmd(
        _get_nc(), in_maps, core_ids=list(range(N_CORES)), trace=want_trace)
    full = np.empty((B, C, HW), np.float32)
    for core in range(N_CORES):
        bi, ch = divmod(core, 2)
        full[bi][:, ch * NQ:(ch + 1) * NQ] = LAST_RESULTS.results[core]["out"]
    return full.reshape(B, C, H, W)


# revision 49
# speedup vs baseline: 1.0047x; 1.0009x over previous
"""AttnBlock for Trainium2, 8 NeuronCores — fp8e4 DoubleRow rewrite (v9).

Sharding: core i = (batch i//2, query-half i%2). Full K/V per core, no
collectives. One program for all cores: odd cores get the image columns
rolled by 2048 (attention is permutation-equivariant over key positions;
GroupNorm stats are order-invariant), so every core computes queries 0..2047
of its (possibly rolled) image.

Math (exact rearrangement of the reference):
  GroupNorm h = scale*x + shift; scale is folded into the fp8 conv WEIGHTS
  on device (w' = fp8(w8 * scale_cin)), so the x->fp8 cast needs no scale
  and runs chunk-by-chunk behind the input DMA, before stats complete.
  shift contributions: k-conv -> constant along m, dropped with bk (softmax
  invariant); v-conv -> v0 = Wv shift + bv contributes Wo v0 to every output
  (softmax rows sum to 1) -> fbias = Wo v0 + bo, pre-added into x in place;
  q-conv -> q0 = Wq shift + bq added in the q epilogue. 1/sqrt(C) is applied
  in the k epilogue. Mg carries 1/GS from the host so group sums emerge as
  [mean, E[x^2]] directly.

Pipeline (cost-model driven; all numbers per core):
  - Phase 1 (~27us, DMA-bound): x streams in 512-col chunks; bn_stats on
    DVE; ONE fp8 copy of x is cast in v-conv stationary layout
    [128, j, mt, ctp, 128] (Act 3/4, Pool 1/4). The k/q convs read their
    MOVING operand from the same tile through a strided view, so the second
    fp8 copy of the old design is gone. A dummy Sqrt at t~0 keeps the
    activation-table load off the stats chain.
  - Convs (~24us): [128,4,256] PSUM ring (4 bufs = all 8 banks), whole-tile
    [128,1024] PSUM->fp8 epilogues alternating Act/DVE (GPSIMD has no PSUM
    port). q convs for the first n-chunk lead; the last three m-chunks'
    k/v epilogues are deferred into attention where the engines have slack.
    The v0/fbias matvecs run mid-conv (off the stats head chain AND off
    the ring tail that the attention pools' bank-reuse waits on).
  - Attention (~73us, jointly exp/PE-bound): per m-chunk, 8 score matmuls
    -> ONE exp(P) [128,4,256] on Act -> PV+den DoubleRow matmuls deferred
    THREE chunks (two for the final chunk, keeping the drain short) so the
    exp engine+sem latency never blocks the PE and the trailing blocks
    spread evenly over the next chunk's first steps. The four
    PV accumulators share 2 PSUM banks as concurrently-open accumulation
    groups (HW start_tensor_calc zeroes only the written region; the sim's
    one-group-per-bank check is skipped). Each chunk's trailing PV pair,
    reciprocal, and softmax divide (one stride-0-broadcast DVE op per half)
    are carried into the next chunk's first steps; proj+residual tails for
    chunk i-1 run at steps 3/5/7 of chunk i. PSUM: s x2 (4 banks) + packed
    PV (2) + den (1) + proj scratch (1).
  - Drain: the freed PV regions double as four independent proj scratches
    (no write-after-read ladder); one wide residual-add and one store.

Known-good invariants learned the hard way:
  - DoubleRow STATIONARY needs its row-pair blocks contiguous in SBUF;
    moving operands tolerate arbitrary strides.
  - Concurrent PSUM accumulation groups in one bank work when they start
    together (PV packing), but a transient group start/stopping while
    another group is mid-accumulation in the same bank corrupts it (the
    den+proj bank-sharing experiment).
  - Pool tiles capture their WAR readers at allocation time: allocate a
    pool generation only after every reader of the previous generation has
    been emitted (the carried-divide race).
"""

import math
import os
import sys

sys.path.insert(0, "/opt/trn_rl_repo")

import numpy as np
import ml_dtypes

import concourse.bacc as bacc
import concourse.bass as bass
import concourse.mybir as mybir
import concourse.tile as tile
from concourse.bass_utils import run_bass_kernel_spmd

F32 = mybir.dt.float32
F32R = mybir.dt.float32r
FP8 = mybir.dt.float8e4
DR = mybir.MatmulPerfMode.DoubleRow
MULT = mybir.AluOpType.mult
ADD = mybir.AluOpType.add
SUB = mybir.AluOpType.subtract
EXP = mybir.ActivationFunctionType.Exp
IDENT = mybir.ActivationFunctionType.Identity
COPY = mybir.ActivationFunctionType.Copy
SQRT = mybir.ActivationFunctionType.Sqrt

B, C, H, W = 4, 512, 64, 64
HW = H * W
G = 32
GS = C // G
NQ = HW // 2
EPS = 1e-5
N_CORES = 8
CT = C // 128
MC = HW // 512          # 8 m-chunks
NC = NQ // 256          # 8 n-chunks
INV_SQRT_C = 1.0 / math.sqrt(C)

LAST_RESULTS = None


def _build():
    nc = bacc.Bacc("TRN2", target_bir_lowering=False, debug=False)

    x_d = nc.dram_tensor("x_img", [C, HW], F32R, kind="ExternalInput").ap()
    w_st = {n: nc.dram_tensor(n, [128, 2, 4, 2, 128], FP8, kind="ExternalInput").ap()
            for n in ("wq8", "wk8", "wo8")}
    wv8m_d = nc.dram_tensor("wv8m", [128, 2, 2, 512], FP8, kind="ExternalInput").ap()
    mg_d = nc.dram_tensor("Mg", [C, G], F32, kind="ExternalInput").ap()
    m2_d = nc.dram_tensor("M2", [G, C], F32, kind="ExternalInput").ap()
    # gamma, beta, bq, bv, bo packed as one [5, C] tensor -> [128, 5, CT] cols
    v5_d = nc.dram_tensor("vec5", [5, C], F32, kind="ExternalInput").ap()
    out_d = nc.dram_tensor("out", [C, NQ], F32, kind="ExternalOutput").ap()

    rx = x_d.rearrange("(t p) m -> p t m", p=128)
    rout = out_d.rearrange("(t p) n -> p t n", p=128)

    with tile.TileContext(nc) as tc:
        with (
            tc.tile_pool(name="singles", bufs=1) as singles,
            tc.tile_pool(name="statp", bufs=3) as statp,
            tc.tile_pool(name="p8p", bufs=6) as p8p,
            tc.tile_pool(name="ao8p", bufs=3) as ao8p,
            tc.tile_pool(name="otp", bufs=3) as otp,
        ):
            # ------ pass 1: x DMA first (the big transfer paces everything);
            # per chunk: bn_stats (DVE only) + x8 cast (Act 3/4, Pool 1/4)
            x_t = singles.tile([128, CT, HW], F32R, tag="x_t")
            stats_all = singles.tile([128, CT, MC + 1, 6], F32, tag="stats_all")
            # Single fp8 copy of x in v-conv stationary layout
            # [128, j, mt, ctp, 128] (ctp pair blocks contiguous, as DoubleRow
            # stationary requires). The k/q convs read their MOVING operand
            # from the same tile through a strided [128, ctp, mt, 128] view.
            x8v = singles.tile([128, 2, HW // 128, 2, 128], FP8, tag="x8v")
            for mc in range(MC):
                ms = slice(mc * 512, (mc + 1) * 512)
                if mc in (0, MC - 1):
                    # split the first chunk (descriptor-generation latency
                    # shouldn't delay first bytes) and the last chunk (its
                    # first-half bn_stats overlap the second half's DMA,
                    # shortening the stats tail; bn_stats are sufficient
                    # statistics so unequal segments aggregate exactly)
                    h0 = slice(mc * 512, mc * 512 + 256)
                    h1 = slice(mc * 512 + 256, (mc + 1) * 512)
                    nc.sync.dma_start(out=x_t[:, :, h0], in_=rx[:, :, h0])
                    nc.sync.dma_start(out=x_t[:, :, h1], in_=rx[:, :, h1])
                else:
                    nc.sync.dma_start(out=x_t[:, :, ms], in_=rx[:, :, ms])
                if mc == MC - 1:
                    h0 = slice(mc * 512, mc * 512 + 256)
                    h1 = slice(mc * 512 + 256, (mc + 1) * 512)
                    for t in range(CT):
                        nc.vector.bn_stats(out=stats_all[:, t, mc, :],
                                           in_=x_t[:, t, h0])
                        nc.vector.bn_stats(out=stats_all[:, t, MC, :],
                                           in_=x_t[:, t, h1])
                else:
                    for t in range(CT):
                        nc.vector.bn_stats(out=stats_all[:, t, mc, :],
                                           in_=x_t[:, t, ms])
                if mc % 2 == 1:
                    m2s = slice((mc - 1) * 512, (mc + 1) * 512)
                    for t in range(CT):
                        dst = x8v[:, t // 2, 4 * mc - 4:4 * mc + 4, t % 2, :]
                        src = x_t[:, t, m2s].rearrange("p (mt m) -> p mt m",
                                                       m=128)
                        if t < 3:
                            nc.scalar.activation(out=dst, in_=src, func=COPY)
                        else:
                            nc.gpsimd.tensor_copy(out=dst, in_=src)

            # ---------------- constants / small loads ----------------
            v5 = singles.tile([128, 5, CT], F32, tag="v5")
            nc.sync.dma_start(out=v5, in_=v5_d.rearrange("v (t p) -> p v t", p=128))
            cols = {n: v5[:, i, :]
                    for i, n in enumerate(("gamma", "beta", "bq", "bv", "bo"))}
            eps_t = singles.tile([G, 1], F32, tag="eps")
            nc.vector.memset(eps_t, EPS)
            # touch Sqrt once at t~0 so its activation table is resident
            # before the GroupNorm stats chain needs it
            warm = singles.tile([1, 1], F32, tag="warm")
            nc.vector.memset(warm, 1.0)
            nc.scalar.activation(out=warm, in_=warm, func=SQRT, bias=0.0,
                                 scale=1.0)
            ones8 = singles.tile([128, 2, 128], FP8, tag="ones8")
            nc.vector.memset(ones8, 1.0)
            Mg = singles.tile([128, CT, G], F32, tag="Mg")
            nc.sync.dma_start(out=Mg, in_=mg_d.rearrange("(t p) g -> p t g", p=128))
            M2 = singles.tile([G, CT, 128], F32, tag="M2")
            nc.sync.dma_start(out=M2, in_=m2_d.rearrange("g (t p) -> g t p", p=128))

            w8 = {}
            for n in ("wq8", "wk8", "wo8"):
                t = singles.tile([128, 2, 4, 2, 128], FP8, tag=n, name=n)
                nc.scalar.dma_start(out=t, in_=w_st[n])
                w8[n] = t
            wv8 = singles.tile([128, 2, 2, 512], FP8, tag="wv8m")
            nc.scalar.dma_start(out=wv8, in_=wv8m_d)

            def x8m(j, blk0, nblk):
                # moving view for k/q convs: [128, ctp, mt, 128]
                return x8v[:, j, blk0:blk0 + nblk, :, :].rearrange(
                    "p mt ctp m -> p ctp mt m")
            mv = statp.tile([128, CT, 2], F32, tag="mv")
            for t in range(CT):
                nc.vector.bn_aggr(out=mv[:, t, :], in_=stats_all[:, t, :, :])
            s_cat = statp.tile([128, CT, 2], F32, tag="s_cat")
            nc.vector.tensor_copy(out=s_cat[:, :, 0:1], in_=mv[:, :, 0:1])
            nc.vector.tensor_tensor(s_cat[:, :, 1:2], mv[:, :, 0:1], mv[:, :, 0:1], MULT)
            nc.vector.tensor_tensor(s_cat[:, :, 1:2], s_cat[:, :, 1:2], mv[:, :, 1:2], ADD)

            k8 = singles.tile([128, 2, HW // 128, 2, 128], FP8, tag="k8")
            vT8 = singles.tile([128, HW // 256, CT, 2, 128], FP8, tag="vT8")
            q8 = singles.tile([128, 2, NC, 2, 256], FP8, tag="q8")

            # ---------------- conv phase: dedicated 6-bank ring ----------------
            with tc.tile_pool(name="ps_cv", bufs=4, space="PSUM") as ps_cv:
                def cvtile(name):
                    return ps_cv.tile([128, 4, 256], F32, tag="cv", name=name)

                # ---- GroupNorm stats -> per-channel scale/shift ----
                # (Mg carries 1/GS from the host, so gsum = [mean_g, E2_g])
                gsum_t = cvtile("gsum")
                gsum_ps = gsum_t.rearrange("p a b -> p (a b)")
                for ct in range(CT):
                    nc.tensor.matmul(gsum_ps[0:G, 0:2], Mg[:, ct, :], s_cat[:, ct, :],
                                     start=(ct == 0), stop=(ct == CT - 1))
                gme = statp.tile([G, 2], F32, tag="gme")
                nc.vector.tensor_copy(out=gme, in_=gsum_ps[0:G, 0:2])
                gvar = statp.tile([G, 1], F32, tag="gvar")
                nc.vector.tensor_tensor(gvar, gme[:, 0:1], gme[:, 0:1], MULT)
                nc.vector.tensor_tensor(gvar, gme[:, 1:2], gvar, SUB)
                grstd = statp.tile([G, 2], F32, tag="grstd")
                nc.scalar.activation(out=gvar, in_=gvar, func=SQRT, bias=eps_t, scale=1.0)
                nc.vector.reciprocal(grstd[:, 0:1], gvar)
                nc.vector.tensor_copy(out=grstd[:, 1:2], in_=gme[:, 0:1])
                # rm: all four [rstd, mean] channel broadcasts in one tile
                rm_pc = statp.tile([128, CT, 2], F32, tag="rm_pc")
                rm_t = cvtile("rm")
                rm_ps = rm_t.rearrange("p a b -> p (a b)")
                for ct in range(CT):
                    nc.tensor.matmul(rm_ps[:, 2 * ct:2 * ct + 2], M2[:, ct, :],
                                     grstd, start=True, stop=True)
                nc.vector.tensor_copy(
                    out=rm_pc,
                    in_=rm_ps[:, 0:2 * CT].rearrange("p (ct two) -> p ct two",
                                                     two=2))
                scale_pc = singles.tile([128, CT], F32, tag="scale_pc")
                shift_pc = singles.tile([128, CT], F32, tag="shift_pc")
                nc.vector.tensor_tensor(scale_pc, cols["gamma"], rm_pc[:, :, 0], MULT)

                # ---- scaled weight copies on Act+DVE (originals stay for
                #      the matvecs, so no WAR chain). q and k first: their
                #      convs lead the PE stream.
                wk8s = singles.tile([128, 2, 4, 2, 128], FP8, tag="wk8s")
                wv8s = singles.tile([128, 2, 2, 512], FP8, tag="wv8s")
                wq8s = singles.tile([128, 2, 4, 2, 128], FP8, tag="wq8s")

                def wscale(dst, src, ct):
                    sc = scale_pc[:, ct:ct + 1]
                    if ct % 2 == 0:
                        nc.scalar.activation(out=dst, in_=src, func=COPY, scale=sc)
                    else:
                        nc.vector.tensor_scalar(dst, src, sc, None, MULT)

                for ct in range(CT):
                    j, p = ct // 2, ct % 2
                    wscale(wq8s[:, j, :, p, :], w8["wq8"][:, j, :, p, :], ct)
                for ct in range(CT):
                    j, p = ct // 2, ct % 2
                    wscale(wk8s[:, j, :, p, :], w8["wk8"][:, j, :, p, :], ct)
                for ct in range(CT):
                    j, p = ct // 2, ct % 2
                    nc.gpsimd.tensor_scalar(wv8s[:, j, p, :], wv8[:, j, p, :],
                                            scale_pc[:, ct:ct + 1], None, MULT)

                nc.vector.tensor_tensor(shift_pc, scale_pc, rm_pc[:, :, 1], MULT)
                nc.vector.tensor_tensor(shift_pc, cols["beta"], shift_pc, SUB)
                shift8 = singles.tile([128, CT, 1], FP8, tag="shift8")
                nc.vector.tensor_scalar_mul(shift8[:, :, 0], shift_pc, 1.0)

                # ---- matvecs on UNscaled weights: q0, v0, fbias ----
                q0col = singles.tile([128, CT], F32, tag="q0col")
                v0col = singles.tile([128, CT], F32, tag="v0col")
                v08 = singles.tile([128, CT, 1], FP8, tag="v08")
                fbias = singles.tile([128, CT], F32, tag="fbias")
                mv_t = cvtile("mv_ps")
                mv_ps = mv_t.rearrange("p a b -> p (a b)")
                for dt in range(CT):
                    for ct in range(CT):
                        nc.tensor.matmul(mv_ps[:, dt:dt + 1],
                                         w8["wq8"][:, ct // 2, dt, ct % 2, :],
                                         shift8[:, ct, :],
                                         start=(ct == 0), stop=(ct == CT - 1))
                nc.vector.tensor_tensor(q0col, mv_ps[:, 0:CT], cols["bq"], ADD)
                # ---- convs; whole-tile [128,1024] epilogues alternate
                #      Act/DVE. q for t=0 first (attention needs q8[0]);
                #      the rest of q after k/v (consumed much later).
                def q_conv(t):
                    for i in range(2):
                        qt = cvtile("qc").rearrange("p a b -> p (a b)") \
                            .rearrange("p (d two n) -> p d two n", d=2, n=256)
                        for d in range(2):
                            dt = 2 * i + d
                            for j in range(2):
                                nc.tensor.matmul(qt[:, d], wq8s[:, j, dt],
                                                 x8m(j, 4 * t, 4),
                                                 start=(j == 0), stop=(j == 1),
                                                 perf_mode=DR)
                        for d in range(2):
                            dt = 2 * i + d
                            dst = q8[:, dt // 2, 2 * t:2 * t + 2, dt % 2, :]
                            # t==3 tiles: both epilogues on Act — the DVE
                            # queue otherwise ends in a serial ladder that
                            # gates attention start (s-bank WAR + the pinned
                            # Exp-table load)
                            if d == 0 or t == 3:
                                nc.scalar.activation(
                                    out=dst, in_=qt[:, d], func=IDENT,
                                    bias=q0col[:, dt:dt + 1], scale=1.0)
                            else:
                                nc.vector.tensor_scalar(
                                    dst, qt[:, d], q0col[:, dt:dt + 1], None, ADD)

                q_conv(0)
                deferred_epis = []

                def k_epi(kt, mc, i):
                    def run():
                        dst = k8[:, i, 4 * mc:4 * mc + 4, :, :]
                        src = kt.rearrange("p d mt m -> p mt d m")
                        if i == 0:
                            nc.scalar.activation(out=dst, in_=src,
                                                 func=COPY, scale=INV_SQRT_C)
                        else:
                            nc.vector.tensor_scalar(dst, src,
                                                    INV_SQRT_C, None, MULT)
                    return run

                def v_epi(vt, h):
                    def run():
                        dst = vT8[:, h, :, :, :]
                        src = vt.rearrange("p g ct m -> p ct g m")
                        if (h % 2 == 1 and h != 15) or h in (6, 14):
                            nc.scalar.activation(out=dst, in_=src, func=COPY)
                        else:
                            nc.vector.tensor_copy(out=dst, in_=src)
                    return run

                def fbias_matvecs():
                    # v0/fbias matvecs mid-conv: late enough to stay off the
                    # stats->wk8s head chain, early enough that the attention
                    # pools' bank-reuse WAR does not wait on them
                    mv_t2 = cvtile("mv_ps2")
                    mv_ps2 = mv_t2.rearrange("p a b -> p (a b)")
                    for dt in range(CT):
                        for ct in range(CT):
                            nc.tensor.matmul(mv_ps2[:, dt:dt + 1],
                                             wv8[:, ct // 2, ct % 2,
                                                 dt * 128:(dt + 1) * 128],
                                             shift8[:, ct, :],
                                             start=(ct == 0),
                                             stop=(ct == CT - 1))
                    nc.vector.tensor_tensor(v0col, mv_ps2[:, 0:CT],
                                            cols["bv"], ADD)
                    nc.vector.tensor_scalar_mul(v08[:, :, 0], v0col, 1.0)
                    mv_t3 = cvtile("mv_ps3")
                    mv_ps3 = mv_t3.rearrange("p a b -> p (a b)")
                    for dt in range(CT):
                        for ct in range(CT):
                            nc.tensor.matmul(mv_ps3[:, dt:dt + 1],
                                             w8["wo8"][:, ct // 2, dt,
                                                       ct % 2, :],
                                             v08[:, ct, :],
                                             start=(ct == 0),
                                             stop=(ct == CT - 1))
                    nc.vector.tensor_tensor(fbias, mv_ps3[:, 0:CT],
                                            cols["bo"], ADD)
                    for ct in range(CT):
                        nc.gpsimd.tensor_scalar(
                            x_t[:, ct, 0:NQ], x_t[:, ct, 0:NQ],
                            fbias[:, ct:ct + 1], None, ADD)

                for mc in range(MC):
                    if mc == 4:
                        fbias_matvecs()
                    for i in range(2):          # dt pair (2i, 2i+1)
                        kt = cvtile("kc").rearrange("p a b -> p (a b)") \
                            .rearrange("p (d mt m) -> p d mt m", d=2, m=128)
                        for d in range(2):
                            dt = 2 * i + d
                            for j in range(2):
                                nc.tensor.matmul(kt[:, d], wk8s[:, j, dt],
                                                 x8m(j, 4 * mc, 4),
                                                 start=(j == 0), stop=(j == 1),
                                                 perf_mode=DR)
                        if mc < MC - 3:
                            k_epi(kt, mc, i)()
                        else:
                            deferred_epis.append(k_epi(kt, mc, i))
                    for h in (2 * mc, 2 * mc + 1):  # msub pair (2h, 2h+1)
                        vt = cvtile("vc").rearrange("p a b -> p (a b)") \
                            .rearrange("p (g ct m) -> p g ct m", g=2, m=128)
                        for gi in range(2):
                            g = 2 * h + gi
                            for j in range(2):
                                nc.tensor.matmul(vt[:, gi], x8v[:, j, g],
                                                 wv8s[:, j],
                                                 start=(j == 0), stop=(j == 1),
                                                 perf_mode=DR)
                        if mc < MC - 3:
                            v_epi(vt, h)()
                        else:
                            deferred_epis.append(v_epi(vt, h))
                for t in range(1, CT):
                    q_conv(t)
                # preload the Exp activation table: this dummy exp DEPENDS on
                # the last q8 epilogue, pinning the (implicit) table load to
                # the conv tail where Act idles — an undepended dummy gets
                # scheduled early and steals Act time from conv epilogues
                nc.scalar.activation(out=warm, in_=q8[0:1, 1, NC - 1, 1, 0:1],
                                     func=EXP)


            # ---------------- attention (s x2 + pv + den/proj = 8 banks) ----
            with (
                tc.tile_pool(name="ps_s", bufs=2, space="PSUM") as ps_s,
                tc.tile_pool(name="ps_pv", bufs=1, space="PSUM") as ps_pv,
                tc.tile_pool(name="ps_o", bufs=1, space="PSUM") as ps_o,
            ):
                pending = None  # (nci, ao8)

                def tail_step(dts, ot, pr):
                    pnci, pao8 = pending
                    for dt in dts:
                        pr_ps = pr[:, dt % 2, :]
                        for j in range(2):
                            nc.tensor.matmul(pr_ps, w8["wo8"][:, j, dt],
                                             pao8[:, 2 * j:2 * j + 2, :],
                                             start=(j == 0), stop=(j == 1),
                                             perf_mode=DR,
                                             skip_group_check=True)
                        nc.vector.tensor_tensor(
                            ot[:, dt, :], pr_ps,
                            x_t[:, dt, pnci * 256:(pnci + 1) * 256], ADD)

                def tail_flush(ot):
                    pnci = pending[0]
                    nc.sync.dma_start(
                        out=rout[:, :, pnci * 256:(pnci + 1) * 256], in_=ot)

                def emit_pv(pm, pp, pv, den_ps):
                    # den first: its stop gates the reciprocal, so retiring
                    # it at block start shortens the divide chain
                    for u in range(2):
                        nc.tensor.matmul(
                            den_ps, ones8, pp[:, 2 * u:2 * u + 2, :],
                            start=(pm == 0 and u == 0),
                            stop=(pm == MC - 1 and u == 1), perf_mode=DR,
                            skip_group_check=True)
                        for ct in range(CT):
                            nc.tensor.matmul(
                                pv[ct], vT8[:, 2 * pm + u, ct],
                                pp[:, 2 * u:2 * u + 2, :],
                                start=(pm == 0 and u == 0),
                                stop=(pm == MC - 1 and u == 1),
                                perf_mode=DR, skip_group_check=True)

                def divide(pnci, prev_pv_all, den_ps):
                    # reciprocal + two half-divides (each half releases its
                    # pv regions and unblocks the matching proj j-step)
                    rec = statp.tile([128, 256], F32, tag="rec", name="rec")
                    nc.vector.reciprocal(rec, den_ps)
                    ao8 = ao8p.tile([128, CT, 256], FP8, tag="ao8", name="ao8")
                    rec_h = bass.AP(rec.tensor, rec.offset,
                                    [rec.ap[0], [0, 2], rec.ap[1]])
                    for hh in range(2):
                        nc.vector.tensor_tensor(
                            ao8[:, 2 * hh:2 * hh + 2, :],
                            prev_pv_all[:, 2 * hh:2 * hh + 2, :], rec_h, MULT)
                    return (pnci, ao8)

                prev = None  # (p_tiles, pv_all, den_ps) of nci-1
                prev_defer = None
                for nci in range(NC):
                    # defer-3 spreads each chunk's trailing PV over three
                    # steps of the next chunk; the last chunk stays defer-2
                    # so the drain keeps only two trailing blocks
                    defer = 2 if nci == NC - 1 else 3
                    p_tiles = []
                    pv_all = den_ps = pv = None
                    ot = otp.tile([128, CT, 256], F32, tag="ot", name="ot") \
                        if prev is not None else None
                    pr = ps_o.tile([128, 2, 256], F32, tag="o", name="pr") \
                        if prev is not None else None
                    for mc in range(MC):
                        s_ps = ps_s.tile([128, 4, 256], F32, tag="s", name="s_ps")
                        for msub in range(4):
                            for j in range(2):
                                nc.tensor.matmul(
                                    s_ps[:, msub, :], k8[:, j, 4 * mc + msub],
                                    q8[:, j, nci],
                                    start=(j == 0), stop=(j == 1), perf_mode=DR)
                        p8 = p8p.tile([128, 4, 256], FP8, tag="p8", name="p8")
                        p_tiles.append(p8)
                        nc.scalar.activation(out=p8, in_=s_ps, func=EXP)
                        if nci == 0 and mc >= 2 and deferred_epis:
                            deferred_epis.pop(0)()
                            if deferred_epis:
                                deferred_epis.pop(0)()
                        if prev is not None and mc < prev_defer:
                            # previous chunk's trailing PV + softmax divide
                            pp_, pva_, den_ = prev
                            emit_pv(MC - prev_defer + mc,
                                    pp_[MC - prev_defer + mc],
                                    [pva_[:, ct, :] for ct in range(CT)], den_)
                            if mc == prev_defer - 1:
                                pending = divide(nci - 1, pva_, den_)
                        if mc == defer:
                            # allocate AFTER the previous generation's readers
                            # (trailing PV + divide) are emitted, so the pool
                            # WAR edges cover them
                            pv_all = ps_pv.tile([128, 4, 256], F32, tag="pva",
                                                name="pva")
                            pv = [pv_all[:, ct, :] for ct in range(CT)]
                            den_ps = ps_pv.tile([128, 256], F32, tag="den",
                                                name="den")
                        if mc >= defer:
                            emit_pv(mc - defer, p_tiles[mc - defer], pv, den_ps)
                        if pending is not None and mc >= 4:
                            tail_step([mc - 4], ot, pr)
                            if mc == MC - 1:
                                tail_flush(ot)
                                pending = None
                    prev = (p_tiles, pv_all, den_ps)
                    prev_defer = defer
                # drain: last chunk's trailing PV, divide, proj, store
                pp_, pva_, den_ = prev
                pvl = [pva_[:, ct, :] for ct in range(CT)]
                emit_pv(MC - 2, pp_[MC - 2], pvl, den_)
                emit_pv(MC - 1, pp_[MC - 1], pvl, den_)
                # final drain pipelined by query-halves: divide, proj,
                # residual and store for queries 0:128 flow while 128:256 is
                # still dividing. Per-dt j-pair order and per-region
                # accumulation order are unchanged.
                pnci = NC - 1
                rec = statp.tile([128, 256], F32, tag="rec", name="rec_f")
                nc.vector.reciprocal(rec, den_)
                ao8 = ao8p.tile([128, CT, 256], FP8, tag="ao8", name="ao8_f")
                ot = otp.tile([128, CT, 256], F32, tag="ot", name="ot_f")
                for qh in range(2):
                    qs = slice(128 * qh, 128 * qh + 128)
                    rec_q = bass.AP(rec.tensor, rec.offset + 128 * qh,
                                    [rec.ap[0], [0, CT], [1, 128]])
                    nc.vector.tensor_tensor(ao8[:, :, qs], pva_[:, :, qs],
                                            rec_q, MULT)
                for qh in range(2):
                    qs = slice(128 * qh, 128 * qh + 128)
                    for dt in range(CT):
                        pr_ps = pva_[:, dt, qs]
                        for j in range(2):
                            nc.tensor.matmul(pr_ps, w8["wo8"][:, j, dt],
                                             ao8[:, 2 * j:2 * j + 2, qs],
                                             start=(j == 0), stop=(j == 1),
                                             perf_mode=DR,
                                             skip_group_check=True)
                    nc.vector.tensor_tensor(
                        ot[:, :, qs], pva_[:, :, qs],
                        x_t[:, :, pnci * 256 + 128 * qh:
                            pnci * 256 + 128 * qh + 128], ADD)
                    nc.sync.dma_start(
                        out=rout[:, :, pnci * 256 + 128 * qh:
                                 pnci * 256 + 128 * qh + 128],
                        in_=ot[:, :, qs])
    nc.finalize()
    return nc


_NC_CACHE = {}


def _get_nc():
    if "nc" not in _NC_CACHE:
        _NC_CACHE["nc"] = _build()
    return _NC_CACHE["nc"]


def _prep_stationary(w):
    # w: [cout, cin] conv weight -> stationary DR layout [p, j, dt, ctp, m]
    wT = np.ascontiguousarray(w.T)                      # [cin, cout]
    arr = wT.reshape(2, 2, 128, 4, 128)                  # [j, ctp, p, dt, m]
    arr = np.transpose(arr, (2, 0, 3, 1, 4))             # [p, j, dt, ctp, m]
    return np.ascontiguousarray(arr).astype(ml_dtypes.float8_e4m3)


def _prep_moving(w):
    # w: [cout, cin] -> moving DR layout [p, j, ctp, cout]
    wT = np.ascontiguousarray(w.T)                      # [cin, cout]
    arr = wT.reshape(2, 2, 128, 512)                     # [j, ctp, p, cout]
    arr = np.transpose(arr, (2, 0, 1, 3))                # [p, j, ctp, cout]
    return np.ascontiguousarray(arr).astype(ml_dtypes.float8_e4m3)


def kernel(**inputs):
    x = np.ascontiguousarray(np.asarray(inputs["x"], dtype=np.float32))
    gamma = np.asarray(inputs["gamma"], np.float32)
    beta = np.asarray(inputs["beta"], np.float32)
    w = {n: np.asarray(inputs[n], np.float32) for n in ("wq", "wk", "wv", "wo")}
    b = {n: np.asarray(inputs[n], np.float32) for n in ("bq", "bk", "bv", "bo")}

    mg_np = np.zeros((C, G), np.float32)
    mg_np[np.arange(C), np.arange(C) // GS] = 1.0 / GS
    common = {
        "Mg": mg_np,
        "M2": np.ascontiguousarray((mg_np != 0).astype(np.float32).T),
        "wq8": _prep_stationary(w["wq"]),
        "wk8": _prep_stationary(w["wk"]),
        "wo8": _prep_stationary(w["wo"]),
        "wv8m": _prep_moving(w["wv"]),
        "vec5": np.ascontiguousarray(
            np.stack([gamma, beta, b["bq"], b["bv"], b["bo"]])),
    }
    in_maps = []
    for core in range(N_CORES):
        bi, ch = divmod(core, 2)
        xi = x[bi].reshape(C, HW)
        if ch:
            xi = np.roll(xi, -NQ, axis=1)
        m = dict(common)
        m["x_img"] = np.ascontiguousarray(xi)
        in_maps.append(m)

    want_trace = bool(int(os.environ.get("KTRACE", "0")))
    if not want_trace:
        os.environ["BASS_NEVER_TRACE"] = "1"
    global LAST_RESULTS
    LAST_RESULTS = run_bass_kernel_spmd(
        _get_nc(), in_maps, core_ids=list(range(N_CORES)), trace=want_trace)
    full = np.empty((B, C, HW), np.float32)
    for core in range(N_CORES):
        bi, ch = divmod(core, 2)
        full[bi][:, ch * NQ:(ch + 1) * NQ] = LAST_RESULTS.results[core]["out"]
    return full.reshape(B, C, H, W)
